# revision 3
# baseline (speedup 1.0000x reference)
# Trainium2 Bass kernel for nn_MEMORY_34986803593776 (scatter_memory).
#
# Math (per sample b):
#   w        = softmax(ck @ mk^T)                             [M]
#   c0       = qa * sigmoid(mem0 @ Wc0 + bc0)                 [DQA]
#   gate     = sigmoid(c0 @ Wm1 + bm1)                        [M*DV]
#   memPre   = mem0 * gate                                    [M*DV]
#   erase    = sig(sig(c0@We+be) + sig(memPre@Wemv+bemv))     [DV]
#   zt       = sig((c0@Wz+bz) + (memPre@Wzmv+bzmv))           [DV]
#   add      = tanh(tanh(zt@Wza+bza) + tanh(memPre@Wamv+bamv))[DV]
#   new      = memPre*(1 - w[m]*erase[dv]) + w[m]*add[dv]     [M,DV]
#
# Sharding: pure data parallel over batch B=16384 across 8 cores (2048/core).
# On-chip: natural [b, f] layout for elementwise; PE-transposed [f, b]
# chunks feed the f-contraction GEMMs. Big elementwise in bf16.
# Softmax for all tiles is hoisted into a prologue (one ACT-table switch).

import numpy as np
import ml_dtypes

B = 16384
M = 64
DV = 64
DK = 64
DQA = 128
F = M * DV  # 4096
N_CORES = 8
B_CORE = B // N_CORES  # 2048

_BUILD_CACHE = {}


def _build(b_core, iters, with_bm1):
    """Build and compile the single-core Bass program."""
    import concourse.tile as tile
    import concourse.bacc as bacc
    import concourse.mybir as mybir
    from concourse import masks
    from contextlib import ExitStack

    f32 = mybir.dt.float32
    bf16 = mybir.dt.bfloat16
    Alu = mybir.AluOpType
    Act = mybir.ActivationFunctionType

    NT = b_core // 256  # tiles of 256 samples
    assert b_core % 256 == 0

    nc = bacc.Bacc("TRN2", target_bir_lowering=False, debug=False,
                   num_devices=N_CORES)

    # ---- DRAM tensors (host-prepped layouts) ----
    d_mem = nc.dram_tensor("mem", (b_core, F), f32, kind="ExternalInput")
    d_qa = nc.dram_tensor("qa", (b_core, DQA), f32, kind="ExternalInput")
    d_ck = nc.dram_tensor("ck", (b_core, DK), f32, kind="ExternalInput")
    d_wc0 = nc.dram_tensor("wc0", (128, 32 * 128), bf16, kind="ExternalInput")
    d_wm1 = nc.dram_tensor("wm1", (128, F), bf16, kind="ExternalInput")
    d_wez = nc.dram_tensor("wez", (128, 32 * 128), bf16, kind="ExternalInput")
    d_wamv = nc.dram_tensor("wamv", (128, 32 * 64), bf16, kind="ExternalInput")
    d_wewz = nc.dram_tensor("wewz", (128, 128), bf16, kind="ExternalInput")
    d_wza = nc.dram_tensor("wza", (DV, DV), bf16, kind="ExternalInput")
    d_mkt = nc.dram_tensor("mkt", (DK, M), bf16, kind="ExternalInput")
    d_bias = nc.dram_tensor("biasv", (128, 8), f32, kind="ExternalInput")
    if with_bm1:
        d_bm1 = nc.dram_tensor("bm1r", (1, F), bf16, kind="ExternalInput")
    d_out = nc.dram_tensor("out", (b_core, F), f32, kind="ExternalOutput")

    mem_r = d_mem.ap().rearrange("(t s p) f -> t p s f", p=128, s=2)
    qa_r = d_qa.ap().rearrange("(t s p) f -> t p s f", p=128, s=2)
    ck_r = d_ck.ap().rearrange("(t s p) f -> t p s f", p=128, s=2)
    out_r = d_out.ap().rearrange("(t s p) f -> t p s f", p=128, s=2)

    with tile.TileContext(nc) as tc:
        with ExitStack() as ctx:
            wpool = ctx.enter_context(tc.tile_pool(name="wpool", bufs=1))
            bigA = ctx.enter_context(tc.tile_pool(name="bigA", bufs=2))
            bigB = ctx.enter_context(tc.tile_pool(name="bigB", bufs=1))
            sml = ctx.enter_context(tc.tile_pool(name="sml", bufs=2))
            pro = ctx.enter_context(tc.tile_pool(name="pro", bufs=1))
            ps_tp = ctx.enter_context(tc.tile_pool(name="ps_tp", bufs=2, space="PSUM"))
            ps_gate = ctx.enter_context(tc.tile_pool(name="ps_gate", bufs=2, space="PSUM"))
            ps_mv = ctx.enter_context(tc.tile_pool(name="ps_mv", bufs=1, space="PSUM"))
            ps_sml = ctx.enter_context(tc.tile_pool(name="ps_sml", bufs=1, space="PSUM"))

            # ---- weights into SBUF (once) ----
            w_c0 = wpool.tile([128, 32, 128], bf16, tag="w_c0")
            nc.sync.dma_start(w_c0[:], d_wc0.ap().rearrange("k (c q) -> k c q", c=32))
            w_m1 = wpool.tile([128, F], bf16, tag="w_m1")
            nc.sync.dma_start(w_m1[:], d_wm1.ap())
            w_ez = wpool.tile([128, 32, 128], bf16, tag="w_ez")
            nc.sync.dma_start(w_ez[:], d_wez.ap().rearrange("k (c q) -> k c q", c=32))
            w_amv = wpool.tile([128, 32, 64], bf16, tag="w_amv")
            nc.sync.dma_start(w_amv[:], d_wamv.ap().rearrange("k (c q) -> k c q", c=32))
            w_ewz = wpool.tile([128, 128], bf16, tag="w_ewz")
            nc.sync.dma_start(w_ewz[:], d_wewz.ap())
            w_za = wpool.tile([DV, DV], bf16, tag="w_za")
            nc.sync.dma_start(w_za[:], d_wza.ap())
            w_mkt = wpool.tile([DK, M], bf16, tag="w_mkt")
            nc.sync.dma_start(w_mkt[:], d_mkt.ap())
            biasv = wpool.tile([128, 8], f32, tag="biasv")
            nc.sync.dma_start(biasv[:], d_bias.ap())
            if with_bm1:
                bm1r = wpool.tile([1, F], bf16, tag="bm1r")
                nc.sync.dma_start(bm1r[:], d_bm1.ap())
                ones1 = wpool.tile([1, 128], bf16, tag="ones1")
                nc.vector.memset(ones1[:], 1.0)
            ident = wpool.tile([128, 128], bf16, tag="ident")
            masks.make_identity(nc, ident[:])

            bc0 = biasv[:, 0:1]
            b_e = biasv[0:64, 1:2]
            b_z = biasv[0:64, 2:3]
            b_emv = biasv[0:64, 3:4]
            b_zmv = biasv[0:64, 4:5]
            b_amv = biasv[0:64, 5:6]
            b_za = biasv[0:64, 6:7]

            def prologue(w_nat_all, w2_all):
                """Softmax for all tiles: w = softmax(ck @ mk^T), natural [b, m].
                Also materialize w2 (pair-duplicated w) for the combine."""
                for t in range(NT):
                    ck = sml.tile([128, 2, DK], bf16, tag="ck")
                    nc.gpsimd.dma_start(ck[:], ck_r[t])
                    ckT = sml.tile([64, 2, 128], bf16, tag="ckT")
                    for s in range(2):
                        tk = ps_tp.tile([128, 128], bf16, tag="tp")
                        nc.tensor.transpose(tk[0:64, :], ck[:, s, :], ident[:])
                        nc.vector.tensor_copy(ckT[:, s, :], tk[0:64, :])
                    lg = ps_sml.tile([128, 2, 64], f32, tag="psml")
                    for s in range(2):
                        nc.tensor.matmul(lg[:, s], ckT[:, s, :], w_mkt[:],
                                         start=True, stop=True)
                    for s in range(2):
                        mx = sml.tile([128, 1], f32, tag="mx")
                        nc.vector.tensor_reduce(mx[:], lg[:, s],
                                                mybir.AxisListType.X,
                                                Alu.max, negate=True)
                        exv = sml.tile([128, 64], f32, tag="exv")
                        nc.scalar.activation(exv[:], lg[:, s], Act.Exp, bias=mx[:])
                        sm = sml.tile([128, 1], f32, tag="sm")
                        nc.vector.tensor_reduce(sm[:], exv[:],
                                                mybir.AxisListType.X, Alu.add)
                        rs = sml.tile([128, 1], f32, tag="rs")
                        nc.vector.reciprocal(rs[:], sm[:])
                        nc.vector.tensor_scalar_mul(w_nat_all[:, t, s, :], exv[:],
                                                    rs[:])
                        # w2[b, 2m+r] = w[b, m]
                        nc.vector.tensor_copy(
                            w2_all[:, t, s, :].rearrange("p (m r) -> p m r", r=2),
                            w_nat_all[:, t, s, :].unsqueeze(2)
                            .broadcast_to([128, 64, 2]))

            def body(t, w_nat_all, w2_all):
                # ---- stage 1: loads (SWDGE cast fp32 -> bf16) ----
                mem = bigA.tile([128, 2, F], bf16, tag="mem")
                nc.gpsimd.dma_start(mem[:], mem_r[t])
                qa = sml.tile([128, 2, DQA], bf16, tag="qa")
                nc.gpsimd.dma_start(qa[:], qa_r[t])

                # ---- stage 2: transpose mem -> memT chunks ([f, b]) ----
                memT = bigB.tile([128, 2, 32, 128], bf16, tag="memT")
                for s in range(2):
                    for cg in range(4):
                        tp = ps_tp.tile([128, 1024], bf16, tag="tp")
                        for c8 in range(8):
                            c = cg * 8 + c8
                            nc.tensor.transpose(
                                tp[:, c8 * 128:(c8 + 1) * 128],
                                mem[:, s, c * 128:(c + 1) * 128], ident[:])
                        nc.scalar.copy(memT[:, s, cg * 8:(cg + 1) * 8, :], tp[:])

                # ---- stage 3: content0 (layout [q, (s,b)]) ----
                c0ps = ps_mv.tile([128, 2, 128], f32, tag="mvc0")
                for c in range(32):
                    nc.tensor.matmul(c0ps[:], w_c0[:, c, :], memT[:, :, c, :],
                                     start=(c == 0), stop=(c == 31))
                c0s = sml.tile([128, 2, 128], bf16, tag="c0s")
                nc.scalar.activation(c0s[:], c0ps[:], Act.Sigmoid, bias=bc0)
                qaT = sml.tile([128, 2, 128], bf16, tag="qaT")
                for s in range(2):
                    tq = ps_tp.tile([128, 128], bf16, tag="tp")
                    nc.tensor.transpose(tq[:], qa[:, s, :], ident[:])
                    nc.vector.tensor_copy(qaT[:, s, :], tq[:])
                c0T = sml.tile([128, 2, 128], bf16, tag="c0T")
                nc.vector.tensor_tensor(c0T[:], c0s[:], qaT[:], op=Alu.mult)

                # ---- stage 4: gate (natural layout [b, f]); psum in bf16 ----
                gate = bigA.tile([128, 2, F], bf16, tag="gate")
                for s in range(2):
                    for h in range(4):
                        gps = ps_gate.tile([128, 1024], bf16, tag="gate")
                        for q in range(2):
                            nsl = slice(h * 1024 + q * 512, h * 1024 + (q + 1) * 512)
                            nc.tensor.matmul(gps[:, q * 512:(q + 1) * 512],
                                             c0T[:, s, :], w_m1[:, nsl],
                                             start=True, stop=not with_bm1)
                            if with_bm1:
                                nc.tensor.matmul(gps[:, q * 512:(q + 1) * 512],
                                                 ones1[:], bm1r[:, nsl],
                                                 start=False, stop=True)
                        nc.scalar.activation(
                            gate[:, s, h * 1024:(h + 1) * 1024], gps[:], Act.Sigmoid)

                # ---- stage 5: memPre = mem * gate ----
                mpre = bigA.tile([128, 2, F], bf16, tag="mpre")
                for s in range(2):
                    nc.vector.tensor_tensor(mpre[:, s], mem[:, s], gate[:, s],
                                            op=Alu.mult)

                # ---- stage 6: transpose memPre -> mpreT ----
                mpreT = bigB.tile([128, 2, 32, 128], bf16, tag="mpreT")
                for s in range(2):
                    for cg in range(4):
                        tp = ps_tp.tile([128, 1024], bf16, tag="tp")
                        for c8 in range(8):
                            c = cg * 8 + c8
                            nc.tensor.transpose(
                                tp[:, c8 * 128:(c8 + 1) * 128],
                                mpre[:, s, c * 128:(c + 1) * 128], ident[:])
                        nc.vector.tensor_copy(mpreT[:, s, cg * 8:(cg + 1) * 8, :],
                                              tp[:])

                # ---- stage 7: mv GEMMs ----
                ez = ps_mv.tile([128, 2, 128], f32, tag="mvez")
                av = ps_mv.tile([64, 2, 128], f32, tag="mvav")
                for c in range(32):
                    nc.tensor.matmul(ez[:], w_ez[:, c, :], mpreT[:, :, c, :],
                                     start=(c == 0), stop=(c == 31))
                for c in range(32):
                    nc.tensor.matmul(av[:], w_amv[:, c, :], mpreT[:, :, c, :],
                                     start=(c == 0), stop=(c == 31))
                emvT = sml.tile([64, 2, 128], bf16, tag="emvT")
                nc.scalar.activation(emvT[:], ez[0:64], Act.Sigmoid, bias=b_emv)
                amvT = sml.tile([64, 2, 128], bf16, tag="amvT")
                nc.scalar.activation(amvT[:], av[:], Act.Tanh, bias=b_amv)

                # ---- stage 8: small epilogue chain ([f, (s,b)]) ----
                wz = ps_sml.tile([128, 2, 128], f32, tag="psml")
                nc.tensor.matmul(wz[:], w_ewz[:], c0T[:], start=True, stop=True)
                ecT = sml.tile([64, 2, 128], bf16, tag="ecT")
                nc.scalar.activation(ecT[:], wz[0:64], Act.Sigmoid, bias=b_e)
                esum = sml.tile([64, 2, 128], bf16, tag="esum")
                nc.vector.tensor_tensor(esum[:], ecT[:], emvT[:], op=Alu.add)
                eT = sml.tile([64, 2, 128], bf16, tag="eT")
                nc.scalar.activation(eT[:], esum[:], Act.Sigmoid)
                zc = sml.tile([64, 2, 128], f32, tag="zc")
                nc.scalar.activation(zc[:], wz[64:128], Act.Identity, bias=b_z)
                zsum = sml.tile([64, 2, 128], f32, tag="zsum")
                nc.vector.scalar_tensor_tensor(zsum[:], ez[64:128], b_zmv, zc[:],
                                               Alu.add, Alu.add)
                ztT = sml.tile([64, 2, 128], bf16, tag="ztT")
                nc.scalar.activation(ztT[:], zsum[:], Act.Sigmoid)
                za = ps_sml.tile([64, 2, 128], f32, tag="psml")
                nc.tensor.matmul(za[:], w_za[:], ztT[:], start=True, stop=True)
                zaT = sml.tile([64, 2, 128], bf16, tag="zaT")
                nc.scalar.activation(zaT[:], za[:], Act.Tanh, bias=b_za)
                asum = sml.tile([64, 2, 128], bf16, tag="asum")
                nc.vector.tensor_tensor(asum[:], zaT[:], amvT[:], op=Alu.add)
                aT = sml.tile([64, 2, 128], bf16, tag="aT")
                nc.scalar.activation(aT[:], asum[:], Act.Tanh)

                # transpose eT/aT -> natural [128(b), s, 64(dv)]
                e_nat = sml.tile([128, 2, 64], bf16, tag="e_nat")
                a_nat = sml.tile([128, 2, 64], bf16, tag="a_nat")
                for s in range(2):
                    te = ps_tp.tile([128, 128], bf16, tag="tp")
                    nc.tensor.transpose(te[:, 0:64], eT[:, s, :],
                                        ident[0:64, 0:64])
                    nc.tensor.transpose(te[:, 64:128], aT[:, s, :],
                                        ident[0:64, 0:64])
                    nc.vector.tensor_copy(e_nat[:, s, :], te[:, 0:64])
                    nc.vector.tensor_copy(a_nat[:, s, :], te[:, 64:128])

                # ---- stage 10: combine  new = mpre + wbig*(abig - mpre*ebig) ----
                out = bigA.tile([128, 2, F], bf16, tag="mem")   # alias mem slots
                scr = bigA.tile([128, 2, F], bf16, tag="gate")  # alias gate slots
                for s in range(2):
                    mp = mpre[:, s].rearrange("p (m d) -> p m d", m=64)
                    t1 = scr[:, s].rearrange("p (m d) -> p m d", m=64)
                    ebig = e_nat[:, s, :].unsqueeze(1).broadcast_to([128, 64, 64])
                    abig = a_nat[:, s, :].unsqueeze(1).broadcast_to([128, 64, 64])
                    # wbig via pair-duplicated w2 so innermost AP step stays 1:
                    # view [p, m, 32, 2]; w2 bcast over the 32 pair groups.
                    w4 = (w2_all[:, t, s, :]
                          .rearrange("p (m r) -> p m r", r=2)
                          .unsqueeze(2).broadcast_to([128, 64, 32, 2]))
                    # P1: t1 = mpre * ebig         (DVE)
                    nc.vector.tensor_tensor(t1, mp, ebig, op=Alu.mult)
                    # P2: t1 = abig - t1           (GPSIMD)
                    nc.gpsimd.tensor_tensor(t1, abig, t1, op=Alu.subtract)
                    # P3: t1 = t1 * wbig           (DVE, 2x via pair trick)
                    t1v = scr[:, s].rearrange("p (m g r) -> p m g r", m=64, r=2)
                    nc.vector.tensor_tensor(t1v, t1v, w4, op=Alu.mult)
                    # P4: out = mpre + t1          (DVE)
                    nc.vector.tensor_tensor(
                        out[:, s].rearrange("p (m d) -> p m d", m=64),
                        mp, t1, op=Alu.add)

                # ---- store (SWDGE cast bf16 -> fp32) ----
                nc.gpsimd.dma_start(out_r[t], out[:])

            def whole():
                w_nat_all = pro.tile([128, NT, 2, 64], bf16, tag="w_nat_all")
                w2_all = pro.tile([128, NT, 2, 128], bf16, tag="w2_all")
                prologue(w_nat_all, w2_all)
                for t in range(NT):
                    body(t, w_nat_all, w2_all)

            if iters == 1:
                whole()
            else:
                with tc.For_i(0, iters, 1):
                    whole()

    nc.compile()
    return nc


def _get_nc(b_core, iters, with_bm1):
    key = (b_core, iters, with_bm1)
    if key not in _BUILD_CACHE:
        _BUILD_CACHE[key] = _build(b_core, iters, with_bm1)
    return _BUILD_CACHE[key]


def _prep_weights(inputs):
    bf = ml_dtypes.bfloat16
    wc0 = np.ascontiguousarray(
        inputs["Wc0"].reshape(32, 128, 128).transpose(1, 0, 2).reshape(128, -1)
    ).astype(bf)
    wez_full = np.concatenate([inputs["Wemv"], inputs["Wzmv"]], axis=1)
    wez = np.ascontiguousarray(
        wez_full.reshape(32, 128, 128).transpose(1, 0, 2).reshape(128, -1)
    ).astype(bf)
    wamv = np.ascontiguousarray(
        inputs["Wamv"].reshape(32, 128, 64).transpose(1, 0, 2).reshape(128, -1)
    ).astype(bf)
    wewz = np.concatenate([inputs["We"], inputs["Wz"]], axis=1).astype(bf)
    wm1 = inputs["Wm1"].astype(bf)
    wza = inputs["Wza"].astype(bf)
    mkt = np.ascontiguousarray(inputs["memory_key"].T).astype(bf)

    biasv = np.zeros((128, 8), np.float32)
    biasv[:, 0] = inputs["bc0"]
    biasv[0:64, 1] = inputs["be"]
    biasv[0:64, 2] = inputs["bz"]
    biasv[0:64, 3] = inputs["bemv"]
    biasv[0:64, 4] = inputs["bzmv"]
    biasv[0:64, 5] = inputs["bamv"]
    biasv[0:64, 6] = inputs["bza"]

    w = dict(wc0=wc0, wm1=wm1, wez=wez, wamv=wamv, wewz=wewz, wza=wza,
             mkt=mkt, biasv=biasv)
    with_bm1 = bool(np.any(inputs["bm1"]))
    if with_bm1:
        w["bm1r"] = inputs["bm1"].reshape(1, F).astype(bf)
    return w, with_bm1


def _make_in_maps(inputs, b_core):
    wdict, _ = _prep_weights(inputs)
    mem = np.ascontiguousarray(inputs["memory_value"].reshape(-1, F))
    qa = np.ascontiguousarray(inputs["control_qa"])
    ck = np.ascontiguousarray(inputs["control_key"])
    in_maps = []
    for c in range(N_CORES):
        sl = slice(c * b_core, (c + 1) * b_core)
        in_maps.append(dict(mem=mem[sl], qa=qa[sl], ck=ck[sl], **wdict))
    return in_maps


def kernel(**inputs):
    from concourse import bass_utils
    inputs = {k: np.asarray(v) for k, v in inputs.items()}
    _, with_bm1 = _prep_weights(inputs)
    nc = _get_nc(B_CORE, 1, with_bm1)
    in_maps = _make_in_maps(inputs, B_CORE)
    res = bass_utils.run_bass_kernel_spmd(nc, in_maps, core_ids=list(range(N_CORES)))
    out = np.concatenate([r["out"] for r in res.results], axis=0)
    return out.reshape(B, M, DV).astype(np.float32)


# revision 14
# speedup vs baseline: 1.2205x; 1.2205x over previous
# Trainium2 Bass kernel for nn_MEMORY_34986803593776 (scatter_memory).
#
# Math (per sample b):
#   w        = softmax(ck @ mk^T)                             [M]
#   c0       = qa * sigmoid(mem0 @ Wc0 + bc0)                 [DQA]
#   gate     = sigmoid(c0 @ Wm1 + bm1)                        [M*DV]
#   memPre   = mem0 * gate                                    [M*DV]
#   erase    = sig(sig(c0@We+be) + sig(memPre@Wemv+bemv))     [DV]
#   zt       = sig((c0@Wz+bz) + (memPre@Wzmv+bzmv))           [DV]
#   add      = tanh(tanh(zt@Wza+bza) + tanh(memPre@Wamv+bamv))[DV]
#   new      = memPre*(1 - w[m]*erase[dv]) + w[m]*add[dv]     [M,DV]
#
# Sharding: pure data parallel over batch B=16384 across 8 cores (2048/core).
# On-chip: natural [b, f] layout for elementwise; PE-transposed [f, b]
# chunks feed the f-contraction GEMMs. Big elementwise in bf16.
# Softmax for all tiles is hoisted into a prologue (one ACT-table switch).

import numpy as np
import ml_dtypes

B = 16384
M = 64
DV = 64
DK = 64
DQA = 128
F = M * DV  # 4096
N_CORES = 8
B_CORE = B // N_CORES  # 2048

_BUILD_CACHE = {}


def _build(b_core, iters, with_bm1):
    """Build and compile the single-core Bass program."""
    import concourse.tile as tile
    import concourse.bacc as bacc
    import concourse.mybir as mybir
    from concourse import masks
    from contextlib import ExitStack

    f32 = mybir.dt.float32
    bf16 = mybir.dt.bfloat16
    Alu = mybir.AluOpType
    Act = mybir.ActivationFunctionType

    NT = b_core // 256  # tiles of 256 samples
    assert b_core % 256 == 0

    nc = bacc.Bacc("TRN2", target_bir_lowering=False, debug=False,
                   num_devices=N_CORES)

    # ---- DRAM tensors (host-prepped layouts) ----
    d_mem = nc.dram_tensor("mem", (b_core, F), f32, kind="ExternalInput")
    d_qa = nc.dram_tensor("qa", (b_core, DQA), f32, kind="ExternalInput")
    d_ck = nc.dram_tensor("ck", (b_core, DK), f32, kind="ExternalInput")
    d_wc0 = nc.dram_tensor("wc0", (128, 32 * 128), bf16, kind="ExternalInput")
    d_wm1 = nc.dram_tensor("wm1", (128, F), bf16, kind="ExternalInput")
    d_wez = nc.dram_tensor("wez", (128, 32 * 128), bf16, kind="ExternalInput")
    d_wamv = nc.dram_tensor("wamv", (128, 32 * 64), bf16, kind="ExternalInput")
    d_wewz = nc.dram_tensor("wewz", (128, 128), bf16, kind="ExternalInput")
    d_wza = nc.dram_tensor("wza", (DV, DV), bf16, kind="ExternalInput")
    d_mkt = nc.dram_tensor("mkt", (DK, M), bf16, kind="ExternalInput")
    d_bias = nc.dram_tensor("biasv", (128, 8), f32, kind="ExternalInput")
    if with_bm1:
        d_bm1 = nc.dram_tensor("bm1r", (1, F), bf16, kind="ExternalInput")
    d_out = nc.dram_tensor("out", (b_core, F), f32, kind="ExternalOutput")

    mem_r = d_mem.ap().rearrange("(t s p) f -> t p s f", p=128, s=2)
    qa_r = d_qa.ap().rearrange("(t s p) f -> t p s f", p=128, s=2)
    ck_r = d_ck.ap().rearrange("(t s p) f -> t p s f", p=128, s=2)
    out_r = d_out.ap().rearrange("(t s p) f -> t p s f", p=128, s=2)

    with tile.TileContext(nc) as tc:
        with ExitStack() as ctx:
            wpool = ctx.enter_context(tc.tile_pool(name="wpool", bufs=1))
            poolmo = ctx.enter_context(tc.tile_pool(name="poolmo", bufs=2))
            scrp = ctx.enter_context(tc.tile_pool(name="scrp", bufs=3))
            bigA = ctx.enter_context(tc.tile_pool(name="bigA", bufs=2))
            bigB = ctx.enter_context(tc.tile_pool(name="bigB", bufs=1))
            sml = ctx.enter_context(tc.tile_pool(name="sml", bufs=2))
            pro = ctx.enter_context(tc.tile_pool(name="pro", bufs=1))
            ps_tp = ctx.enter_context(tc.tile_pool(name="ps_tp", bufs=2, space="PSUM"))
            ps_gate = ctx.enter_context(tc.tile_pool(name="ps_gate", bufs=2, space="PSUM"))
            ps_mv = ctx.enter_context(tc.tile_pool(name="ps_mv", bufs=1, space="PSUM"))
            ps_sml = ctx.enter_context(tc.tile_pool(name="ps_sml", bufs=1, space="PSUM"))

            # ---- weights into SBUF (once) ----
            w_c0 = wpool.tile([128, 32, 128], bf16, tag="w_c0")
            nc.sync.dma_start(w_c0[:], d_wc0.ap().rearrange("k (c q) -> k c q", c=32))
            w_m1 = wpool.tile([128, F], bf16, tag="w_m1")
            nc.sync.dma_start(w_m1[:], d_wm1.ap())
            w_ez = wpool.tile([128, 32, 128], bf16, tag="w_ez")
            nc.sync.dma_start(w_ez[:], d_wez.ap().rearrange("k (c q) -> k c q", c=32))
            w_amv = wpool.tile([128, 32, 64], bf16, tag="w_amv")
            nc.sync.dma_start(w_amv[:], d_wamv.ap().rearrange("k (c q) -> k c q", c=32))
            w_ewz = wpool.tile([128, 128], bf16, tag="w_ewz")
            nc.sync.dma_start(w_ewz[:], d_wewz.ap())
            w_za = wpool.tile([DV, DV], bf16, tag="w_za")
            nc.sync.dma_start(w_za[:], d_wza.ap())
            w_mkt = wpool.tile([DK, M], bf16, tag="w_mkt")
            nc.sync.dma_start(w_mkt[:], d_mkt.ap())
            biasv = wpool.tile([128, 8], f32, tag="biasv")
            nc.sync.dma_start(biasv[:], d_bias.ap())
            if with_bm1:
                bm1r = wpool.tile([1, F], bf16, tag="bm1r")
                nc.sync.dma_start(bm1r[:], d_bm1.ap())
                ones1 = wpool.tile([1, 128], bf16, tag="ones1")
                nc.vector.memset(ones1[:], 1.0)
            ident = wpool.tile([128, 128], bf16, tag="ident")
            masks.make_identity(nc, ident[:])

            bc0 = biasv[:, 0:1]
            b_e = biasv[0:64, 1:2]
            b_z = biasv[0:64, 2:3]
            b_emv = biasv[0:64, 3:4]
            b_zmv = biasv[0:64, 4:5]
            b_amv = biasv[0:64, 5:6]
            b_za = biasv[0:64, 6:7]

            def prologue(w_nat_all, w2_all):
                """Softmax for all tiles: w = softmax(ck @ mk^T), natural [b, m].
                Also materialize w2 (pair-duplicated w) for the combine."""
                for t in range(NT):
                    ck = sml.tile([128, 2, DK], bf16, tag="ck")
                    nc.gpsimd.dma_start(ck[:], ck_r[t])
                    ckT = sml.tile([64, 2, 128], bf16, tag="ckT")
                    for s in range(2):
                        tk = ps_tp.tile([128, 128], bf16, tag="tp")
                        nc.tensor.transpose(tk[0:64, :], ck[:, s, :], ident[:])
                        nc.vector.tensor_copy(ckT[:, s, :], tk[0:64, :])
                    lg = ps_sml.tile([128, 2, 64], f32, tag="psml")
                    for s in range(2):
                        nc.tensor.matmul(lg[:, s], ckT[:, s, :], w_mkt[:],
                                         start=True, stop=True)
                    for s in range(2):
                        mx = sml.tile([128, 1], f32, tag="mx")
                        nc.vector.tensor_reduce(mx[:], lg[:, s],
                                                mybir.AxisListType.X,
                                                Alu.max, negate=True)
                        exv = sml.tile([128, 64], f32, tag="exv")
                        nc.scalar.activation(exv[:], lg[:, s], Act.Exp, bias=mx[:])
                        sm = sml.tile([128, 1], f32, tag="sm")
                        nc.vector.tensor_reduce(sm[:], exv[:],
                                                mybir.AxisListType.X, Alu.add)
                        rs = sml.tile([128, 1], f32, tag="rs")
                        nc.vector.reciprocal(rs[:], sm[:])
                        nc.vector.tensor_scalar_mul(w_nat_all[:, t, s, :], exv[:],
                                                    rs[:])
                        # w2[b, 2m+r] = w[b, m]
                        nc.vector.tensor_copy(
                            w2_all[:, t, s, :].rearrange("p (m r) -> p m r", r=2),
                            w_nat_all[:, t, s, :].unsqueeze(2)
                            .broadcast_to([128, 64, 2]))

            def body(t, w_nat_all, w2_all):
                # ---- stage 1: loads (SWDGE cast fp32 -> bf16) ----
                mem = poolmo.tile([128, 2, F], bf16, tag="mem")
                nc.gpsimd.dma_start(mem[:], mem_r[t])
                qa = sml.tile([128, 2, DQA], bf16, tag="qa")
                nc.gpsimd.dma_start(qa[:], qa_r[t])

                # ---- stage 2: transpose mem -> memT chunks ([f, b]) ----
                memT = bigB.tile([128, 2, 32, 128], bf16, tag="memT")
                for s in range(2):
                    for cg in range(4):
                        tp = ps_tp.tile([128, 1024], bf16, tag="tp")
                        for c8 in range(8):
                            c = cg * 8 + c8
                            nc.tensor.transpose(
                                tp[:, c8 * 128:(c8 + 1) * 128],
                                mem[:, s, c * 128:(c + 1) * 128], ident[:])
                        nc.scalar.copy(memT[:, s, cg * 8:(cg + 1) * 8, :], tp[:])

                # ---- stage 3: content0 (layout [q, (s,b)]) ----
                c0ps = ps_mv.tile([128, 2, 128], f32, tag="mvc0")
                for c in range(32):
                    nc.tensor.matmul(c0ps[:], w_c0[:, c, :], memT[:, :, c, :],
                                     start=(c == 0), stop=(c == 31))
                c0s = sml.tile([128, 2, 128], bf16, tag="c0s")
                nc.scalar.activation(c0s[:], c0ps[:], Act.Sigmoid, bias=bc0)
                qaT = sml.tile([128, 2, 128], bf16, tag="qaT")
                for s in range(2):
                    tq = ps_tp.tile([128, 128], bf16, tag="tp")
                    nc.tensor.transpose(tq[:], qa[:, s, :], ident[:])
                    nc.vector.tensor_copy(qaT[:, s, :], tq[:])
                c0T = sml.tile([128, 2, 128], bf16, tag="c0T")
                nc.vector.tensor_tensor(c0T[:], c0s[:], qaT[:], op=Alu.mult)

                # ---- stage 4: gate (natural layout [b, f]) ----
                gate = bigA.tile([128, 2, F], bf16, tag="gate")
                for s in range(2):
                    for h in range(8):
                        gps = ps_gate.tile([128, 512], f32, tag="gate")
                        nsl = slice(h * 512, (h + 1) * 512)
                        nc.tensor.matmul(gps[:], c0T[:, s, :], w_m1[:, nsl],
                                         start=True, stop=not with_bm1)
                        if with_bm1:
                            nc.tensor.matmul(gps[:], ones1[:], bm1r[:, nsl],
                                             start=False, stop=True)
                        nc.scalar.activation(gate[:, s, nsl], gps[:], Act.Sigmoid)

                # ---- stage 5: memPre = mem * gate ----
                mpre = bigA.tile([128, 2, F], bf16, tag="mpre")
                for s in range(2):
                    nc.vector.tensor_tensor(mpre[:, s], mem[:, s], gate[:, s],
                                            op=Alu.mult)

                # ---- stage 6: transpose memPre -> mpreT ----
                mpreT = bigB.tile([128, 2, 32, 128], bf16, tag="mpreT")
                for s in range(2):
                    for cg in range(4):
                        tp = ps_tp.tile([128, 1024], bf16, tag="tp")
                        for c8 in range(8):
                            c = cg * 8 + c8
                            nc.tensor.transpose(
                                tp[:, c8 * 128:(c8 + 1) * 128],
                                mpre[:, s, c * 128:(c + 1) * 128], ident[:])
                        if cg % 2 == 0:
                            nc.vector.tensor_copy(
                                mpreT[:, s, cg * 8:(cg + 1) * 8, :], tp[:])
                        else:
                            nc.scalar.copy(
                                mpreT[:, s, cg * 8:(cg + 1) * 8, :], tp[:])

                # ---- stage 7: mv GEMMs ----
                ez = ps_mv.tile([128, 2, 128], f32, tag="mvez")
                av = ps_mv.tile([64, 2, 128], f32, tag="mvav")
                for c in range(32):
                    nc.tensor.matmul(ez[:], w_ez[:, c, :], mpreT[:, :, c, :],
                                     start=(c == 0), stop=(c == 31))
                for c in range(32):
                    nc.tensor.matmul(av[:], w_amv[:, c, :], mpreT[:, :, c, :],
                                     start=(c == 0), stop=(c == 31))
                emvT = sml.tile([64, 2, 128], bf16, tag="emvT")
                nc.scalar.activation(emvT[:], ez[0:64], Act.Sigmoid, bias=b_emv)
                amvT = sml.tile([64, 2, 128], bf16, tag="amvT")
                nc.scalar.activation(amvT[:], av[:], Act.Tanh, bias=b_amv)
                # drain zmv out of psum early so ez frees for the next tile
                zmv = sml.tile([64, 2, 128], bf16, tag="zmv")
                nc.scalar.activation(zmv[:], ez[64:128], Act.Identity, bias=b_zmv)

                # ---- stage 8: small epilogue chain ([f, (s,b)]) ----
                wz = ps_sml.tile([128, 2, 128], f32, tag="psml")
                nc.tensor.matmul(wz[:], w_ewz[:], c0T[:], start=True, stop=True)
                ecT = sml.tile([64, 2, 128], bf16, tag="ecT")
                nc.scalar.activation(ecT[:], wz[0:64], Act.Sigmoid, bias=b_e)
                esum = sml.tile([64, 2, 128], bf16, tag="esum")
                nc.vector.tensor_tensor(esum[:], ecT[:], emvT[:], op=Alu.add)
                eT = sml.tile([64, 2, 128], bf16, tag="eT")
                nc.scalar.activation(eT[:], esum[:], Act.Sigmoid)
                zc = sml.tile([64, 2, 128], bf16, tag="zc")
                nc.scalar.activation(zc[:], wz[64:128], Act.Identity, bias=b_z)
                zsum = sml.tile([64, 2, 128], bf16, tag="zsum")
                nc.vector.tensor_tensor(zsum[:], zmv[:], zc[:], op=Alu.add)
                ztT = sml.tile([64, 2, 128], bf16, tag="ztT")
                nc.scalar.activation(ztT[:], zsum[:], Act.Sigmoid)
                za = ps_sml.tile([64, 2, 128], f32, tag="psml")
                nc.tensor.matmul(za[:], w_za[:], ztT[:], start=True, stop=True)
                zaT = sml.tile([64, 2, 128], bf16, tag="zaT")
                nc.scalar.activation(zaT[:], za[:], Act.Tanh, bias=b_za)
                asum = sml.tile([64, 2, 128], bf16, tag="asum")
                nc.vector.tensor_tensor(asum[:], zaT[:], amvT[:], op=Alu.add)
                aT = sml.tile([64, 2, 128], bf16, tag="aT")
                nc.scalar.activation(aT[:], asum[:], Act.Tanh)

                # transpose eT/aT -> natural [128(b), s, 64(dv)]
                e_nat = sml.tile([128, 2, 64], bf16, tag="e_nat")
                a_nat = sml.tile([128, 2, 64], bf16, tag="a_nat")
                for s in range(2):
                    te = ps_tp.tile([128, 128], bf16, tag="tp")
                    nc.tensor.transpose(te[:, 0:64], eT[:, s, :],
                                        ident[0:64, 0:64])
                    nc.tensor.transpose(te[:, 64:128], aT[:, s, :],
                                        ident[0:64, 0:64])
                    nc.vector.tensor_copy(e_nat[:, s, :], te[:, 0:64])
                    nc.vector.tensor_copy(a_nat[:, s, :], te[:, 64:128])

                # ---- stage 10: combine  new = mpre + wbig*(abig - mpre*ebig) ----
                out = poolmo.tile([128, 2, F], bf16, tag="mem")  # alias mem slots
                for s in range(2):
                    scr = scrp.tile([128, F], bf16, tag="scr")
                    mp = mpre[:, s].rearrange("p (m d) -> p m d", m=64)
                    t1 = scr[:].rearrange("p (m d) -> p m d", m=64)
                    ebig = e_nat[:, s, :].unsqueeze(1).broadcast_to([128, 64, 64])
                    abig = a_nat[:, s, :].unsqueeze(1).broadcast_to([128, 64, 64])
                    # wbig via pair-duplicated w2 so innermost AP step stays 1:
                    # view [p, m, 32, 2]; w2 bcast over the 32 pair groups.
                    w4 = (w2_all[:, t, s, :]
                          .rearrange("p (m r) -> p m r", r=2)
                          .unsqueeze(2).broadcast_to([128, 64, 32, 2]))
                    # P1: t1 = mpre * ebig         (DVE)
                    nc.vector.tensor_tensor(t1, mp, ebig, op=Alu.mult)
                    # P2: t1 = abig - t1           (DVE)
                    nc.vector.tensor_tensor(t1, abig, t1, op=Alu.subtract)
                    # P3: t1 = t1 * wbig           (DVE, 2x via pair trick)
                    t1v = scr[:].rearrange("p (m g r) -> p m g r", m=64, r=2)
                    nc.vector.tensor_tensor(t1v, t1v, w4, op=Alu.mult)
                    # P4: out = mpre + t1          (DVE)
                    nc.vector.tensor_tensor(
                        out[:, s].rearrange("p (m d) -> p m d", m=64),
                        mp, t1, op=Alu.add)

                # ---- store (SWDGE cast bf16 -> fp32) ----
                nc.gpsimd.dma_start(out_r[t], out[:])

            def whole():
                w_nat_all = pro.tile([128, NT, 2, 64], bf16, tag="w_nat_all")
                w2_all = pro.tile([128, NT, 2, 128], bf16, tag="w2_all")
                prologue(w_nat_all, w2_all)
                for t in range(NT):
                    body(t, w_nat_all, w2_all)

            if iters == 1:
                whole()
            else:
                with tc.For_i(0, iters, 1):
                    whole()

    nc.compile()
    return nc


def _get_nc(b_core, iters, with_bm1):
    key = (b_core, iters, with_bm1)
    if key not in _BUILD_CACHE:
        _BUILD_CACHE[key] = _build(b_core, iters, with_bm1)
    return _BUILD_CACHE[key]


def _prep_weights(inputs):
    bf = ml_dtypes.bfloat16
    wc0 = np.ascontiguousarray(
        inputs["Wc0"].reshape(32, 128, 128).transpose(1, 0, 2).reshape(128, -1)
    ).astype(bf)
    wez_full = np.concatenate([inputs["Wemv"], inputs["Wzmv"]], axis=1)
    wez = np.ascontiguousarray(
        wez_full.reshape(32, 128, 128).transpose(1, 0, 2).reshape(128, -1)
    ).astype(bf)
    wamv = np.ascontiguousarray(
        inputs["Wamv"].reshape(32, 128, 64).transpose(1, 0, 2).reshape(128, -1)
    ).astype(bf)
    wewz = np.concatenate([inputs["We"], inputs["Wz"]], axis=1).astype(bf)
    wm1 = inputs["Wm1"].astype(bf)
    wza = inputs["Wza"].astype(bf)
    mkt = np.ascontiguousarray(inputs["memory_key"].T).astype(bf)

    biasv = np.zeros((128, 8), np.float32)
    biasv[:, 0] = inputs["bc0"]
    biasv[0:64, 1] = inputs["be"]
    biasv[0:64, 2] = inputs["bz"]
    biasv[0:64, 3] = inputs["bemv"]
    biasv[0:64, 4] = inputs["bzmv"]
    biasv[0:64, 5] = inputs["bamv"]
    biasv[0:64, 6] = inputs["bza"]

    w = dict(wc0=wc0, wm1=wm1, wez=wez, wamv=wamv, wewz=wewz, wza=wza,
             mkt=mkt, biasv=biasv)
    with_bm1 = bool(np.any(inputs["bm1"]))
    if with_bm1:
        w["bm1r"] = inputs["bm1"].reshape(1, F).astype(bf)
    return w, with_bm1


def _make_in_maps(inputs, b_core):
    wdict, _ = _prep_weights(inputs)
    mem = np.ascontiguousarray(inputs["memory_value"].reshape(-1, F))
    qa = np.ascontiguousarray(inputs["control_qa"])
    ck = np.ascontiguousarray(inputs["control_key"])
    in_maps = []
    for c in range(N_CORES):
        sl = slice(c * b_core, (c + 1) * b_core)
        in_maps.append(dict(mem=mem[sl], qa=qa[sl], ck=ck[sl], **wdict))
    return in_maps


def kernel(**inputs):
    from concourse import bass_utils
    inputs = {k: np.asarray(v) for k, v in inputs.items()}
    _, with_bm1 = _prep_weights(inputs)
    nc = _get_nc(B_CORE, 1, with_bm1)
    in_maps = _make_in_maps(inputs, B_CORE)
    res = bass_utils.run_bass_kernel_spmd(nc, in_maps, core_ids=list(range(N_CORES)))
    out = np.concatenate([r["out"] for r in res.results], axis=0)
    return out.reshape(B, M, DV).astype(np.float32)


# revision 27
# speedup vs baseline: 1.8816x; 1.5416x over previous
# Trainium2 Bass kernel for nn_MEMORY_34986803593776 (scatter_memory).
#
# Math (per sample b):
#   w        = softmax(ck @ mk^T)                             [M]
#   c0       = qa * sigmoid(mem0 @ Wc0 + bc0)                 [DQA]
#   gate     = sigmoid(c0 @ Wm1 + bm1)                        [M*DV]
#   memPre   = mem0 * gate                                    [M*DV]
#   erase    = sig(sig(c0@We+be) + sig(memPre@Wemv+bemv))     [DV]
#   zt       = sig((c0@Wz+bz) + (memPre@Wzmv+bzmv))           [DV]
#   add      = tanh(tanh(zt@Wza+bza) + tanh(memPre@Wamv+bamv))[DV]
#   new      = memPre*(1 - w[m]*erase[dv]) + w[m]*add[dv]     [M,DV]
#
# Sharding: pure data parallel over batch B=16384 across 8 cores (2048/core).
# On-chip: natural [b, f] layout for elementwise; PE-transposed [f, b]
# chunks feed the f-contraction GEMMs. Big elementwise in bf16.
# Softmax for all tiles is hoisted into a prologue (one ACT-table switch).

import numpy as np
import ml_dtypes

B = 16384
M = 64
DV = 64
DK = 64
DQA = 128
F = M * DV  # 4096
N_CORES = 8
B_CORE = B // N_CORES  # 2048

_BUILD_CACHE = {}


def _build(b_core, iters, with_bm1):
    """Build and compile the single-core Bass program."""
    import concourse.tile as tile
    import concourse.bacc as bacc
    import concourse.mybir as mybir
    from concourse import masks
    from contextlib import ExitStack

    f32 = mybir.dt.float32
    bf16 = mybir.dt.bfloat16
    Alu = mybir.AluOpType
    Act = mybir.ActivationFunctionType

    NT = b_core // 256  # tiles of 256 samples
    assert b_core % 256 == 0

    nc = bacc.Bacc("TRN2", target_bir_lowering=False, debug=False,
                   num_devices=N_CORES)

    # ---- DRAM tensors (host-prepped layouts) ----
    d_mem = nc.dram_tensor("mem", (b_core, F), f32, kind="ExternalInput")
    d_qa = nc.dram_tensor("qa", (b_core, DQA), f32, kind="ExternalInput")
    d_ck = nc.dram_tensor("ck", (b_core, DK), f32, kind="ExternalInput")
    d_wc0 = nc.dram_tensor("wc0", (128, 32 * 128), bf16, kind="ExternalInput")
    d_wm1 = nc.dram_tensor("wm1", (128, F), bf16, kind="ExternalInput")
    d_wez = nc.dram_tensor("wez", (128, 32 * 128), bf16, kind="ExternalInput")
    d_wamv = nc.dram_tensor("wamv", (128, 32 * 64), bf16, kind="ExternalInput")
    d_wewz = nc.dram_tensor("wewz", (128, 128), bf16, kind="ExternalInput")
    d_wza = nc.dram_tensor("wza", (DV, DV), bf16, kind="ExternalInput")
    d_mkt = nc.dram_tensor("mkt", (DK, M), bf16, kind="ExternalInput")
    d_bias = nc.dram_tensor("biasv", (128, 8), f32, kind="ExternalInput")
    if with_bm1:
        d_bm1 = nc.dram_tensor("bm1r", (1, F), bf16, kind="ExternalInput")
    d_out = nc.dram_tensor("out", (b_core, F), f32, kind="ExternalOutput")

    mem_r = d_mem.ap().rearrange("(t s p) f -> t p s f", p=128, s=2)
    qa_r = d_qa.ap().rearrange("(t s p) f -> t p s f", p=128, s=2)
    ck_r = d_ck.ap().rearrange("(t s p) f -> t p s f", p=128, s=2)
    out_r = d_out.ap().rearrange("(t s p) f -> t p s f", p=128, s=2)

    with tile.TileContext(nc) as tc:
        with ExitStack() as ctx:
            wpool = ctx.enter_context(tc.tile_pool(name="wpool", bufs=1))
            poolmo = ctx.enter_context(tc.tile_pool(name="poolmo", bufs=3))
            scrp = ctx.enter_context(tc.tile_pool(name="scrp", bufs=2))
            bigA = ctx.enter_context(tc.tile_pool(name="bigA", bufs=2))
            bigB = ctx.enter_context(tc.tile_pool(name="bigB", bufs=1))
            sml = ctx.enter_context(tc.tile_pool(name="sml", bufs=2))
            pro = ctx.enter_context(tc.tile_pool(name="pro", bufs=1))
            ps_tp = ctx.enter_context(tc.tile_pool(name="ps_tp", bufs=2, space="PSUM"))
            ps_gate = ctx.enter_context(tc.tile_pool(name="ps_gate", bufs=2, space="PSUM"))
            ps_sml = ctx.enter_context(tc.tile_pool(name="ps_sml", bufs=1, space="PSUM"))
            ps_mv = ctx.enter_context(tc.tile_pool(name="ps_mv", bufs=1, space="PSUM"))

            # ---- weights into SBUF (once) ----
            w_c0 = wpool.tile([128, 32, 128], bf16, tag="w_c0")
            nc.sync.dma_start(w_c0[:], d_wc0.ap().rearrange("k (c q) -> k c q", c=32))
            w_m1 = wpool.tile([128, F], bf16, tag="w_m1")
            nc.sync.dma_start(w_m1[:], d_wm1.ap())
            w_ez = wpool.tile([128, 32, 128], bf16, tag="w_ez")
            nc.sync.dma_start(w_ez[:], d_wez.ap().rearrange("k (c q) -> k c q", c=32))
            w_amv = wpool.tile([128, 32, 64], bf16, tag="w_amv")
            nc.sync.dma_start(w_amv[:], d_wamv.ap().rearrange("k (c q) -> k c q", c=32))
            w_ewz = wpool.tile([128, 128], bf16, tag="w_ewz")
            nc.sync.dma_start(w_ewz[:], d_wewz.ap())
            w_za = wpool.tile([DV, DV], bf16, tag="w_za")
            nc.sync.dma_start(w_za[:], d_wza.ap())
            w_mkt = wpool.tile([DK, M], bf16, tag="w_mkt")
            nc.sync.dma_start(w_mkt[:], d_mkt.ap())
            biasv = wpool.tile([128, 8], f32, tag="biasv")
            nc.sync.dma_start(biasv[:], d_bias.ap())
            if with_bm1:
                bm1r = wpool.tile([1, F], bf16, tag="bm1r")
                nc.sync.dma_start(bm1r[:], d_bm1.ap())
                ones1 = wpool.tile([1, 128], bf16, tag="ones1")
                nc.vector.memset(ones1[:], 1.0)
            ident = wpool.tile([128, 128], bf16, tag="ident")
            masks.make_identity(nc, ident[:])

            bc0 = biasv[:, 0:1]
            b_e = biasv[0:64, 1:2]
            b_z = biasv[0:64, 2:3]
            b_emv = biasv[0:64, 3:4]
            b_zmv = biasv[0:64, 4:5]
            b_amv = biasv[0:64, 5:6]
            b_za = biasv[0:64, 6:7]

            def prologue(w_nat_all):
                """Softmax for all tiles: w = softmax(ck @ mk^T), natural [b, m].
                Also materialize w2 (pair-duplicated w) for the combine."""
                for t in range(NT):
                    ck = sml.tile([128, 2, DK], bf16, tag="qa")
                    nc.gpsimd.dma_start(ck[:], ck_r[t])
                    ckT = sml.tile([64, 2, 128], bf16, tag="ecT")
                    for s in range(2):
                        tk = ps_tp.tile([128, 128], bf16, tag="tp")
                        nc.tensor.transpose(tk[0:64, :], ck[:, s, :], ident[:])
                        nc.vector.tensor_copy(ckT[:, s, :], tk[0:64, :])
                    lg = ps_sml.tile([128, 2, 64], f32, tag="psml")
                    for s in range(2):
                        nc.tensor.matmul(lg[:, s], ckT[:, s, :], w_mkt[:],
                                         start=True, stop=True)
                    for s in range(2):
                        mx = sml.tile([128, 1], f32, tag="mx")
                        nc.vector.tensor_reduce(mx[:], lg[:, s],
                                                mybir.AxisListType.X,
                                                Alu.max, negate=True)
                        exv = sml.tile([128, 64], f32, tag="exv")
                        nc.scalar.activation(exv[:], lg[:, s], Act.Exp, bias=mx[:])
                        sm = sml.tile([128, 1], f32, tag="sm")
                        nc.vector.tensor_reduce(sm[:], exv[:],
                                                mybir.AxisListType.X, Alu.add)
                        nc.vector.reciprocal(sm[:], sm[:])
                        nc.vector.tensor_scalar_mul(w_nat_all[:, t, s, :], exv[:],
                                                    sm[:])


            def load_tile(t):
                qa = sml.tile([128, 2, DQA], bf16, tag="qa")
                nc.gpsimd.dma_start(qa[:], qa_r[t])
                mem = poolmo.tile([128, 2, F], bf16, tag="mem")
                nc.gpsimd.dma_start(mem[:], mem_r[t])
                return mem, qa

            def stage_a1(t, loaded):
                mem, qa = loaded

                # ---- stage 2: transpose mem -> memT chunks ([f, b]) ----
                memT = bigB.tile([128, 2, 32, 128], bf16, tag="memT")
                for s in range(2):
                    for cg in range(4):
                        tp = ps_tp.tile([128, 1024], bf16, tag="tp")
                        for c8 in range(8):
                            c = cg * 8 + c8
                            nc.tensor.transpose(
                                tp[:, c8 * 128:(c8 + 1) * 128],
                                mem[:, s, c * 128:(c + 1) * 128], ident[:])
                        nc.scalar.copy(memT[:, s, cg * 8:(cg + 1) * 8, :], tp[:])

                # ---- stage 3: content0 (layout [q, (s,b)]) ----
                c0ps = ps_mv.tile([128, 2, 128], f32, tag="mvc0")
                for c in range(32):
                    nc.tensor.matmul(c0ps[:], w_c0[:, c, :], memT[:, :, c, :],
                                     start=(c == 0), stop=(c == 31))
                c0s = sml.tile([128, 2, 128], bf16, tag="c0s")
                nc.scalar.activation(c0s[:], c0ps[:], Act.Sigmoid, bias=bc0)
                qaT = sml.tile([128, 2, 128], bf16, tag="qaT")
                for s in range(2):
                    tq = ps_tp.tile([128, 128], bf16, tag="tp")
                    nc.tensor.transpose(tq[:], qa[:, s, :], ident[:])
                    nc.vector.tensor_copy(qaT[:, s, :], tq[:])
                return dict(mem=mem, memT=memT, c0s=c0s, qaT=qaT)

            def stage_a2(t, st):
                c0T = sml.tile([128, 2, 128], bf16, tag="c0T")
                nc.vector.tensor_tensor(c0T[:], st["c0s"][:], st["qaT"][:],
                                        op=Alu.mult)

                # ---- stage 4: gate (natural layout [b, f]) ----
                gate = bigA.tile([128, 2, F], bf16, tag="gate")
                for s in range(2):
                    for h in range(8):
                        gps = ps_gate.tile([128, 512], f32, tag="gate")
                        nsl = slice(h * 512, (h + 1) * 512)
                        nc.tensor.matmul(gps[:], c0T[:, s, :], w_m1[:, nsl],
                                         start=True, stop=not with_bm1)
                        if with_bm1:
                            nc.tensor.matmul(gps[:], ones1[:], bm1r[:, nsl],
                                             start=False, stop=True)
                        nc.scalar.activation(gate[:, s, nsl], gps[:], Act.Sigmoid)

                st["c0T"] = c0T
                st["gate"] = gate
                return st

            def stage_b(t, w_nat_all, st):
                mem, memT, c0T, gate = st["mem"], st["memT"], st["c0T"], st["gate"]
                # ---- stage 5: memPre = mem * gate ----
                mpre = bigA.tile([128, 2, F], bf16, tag="mpre")
                for s in range(2):
                    nc.vector.tensor_tensor(mpre[:, s], mem[:, s], gate[:, s],
                                            op=Alu.mult)

                # ---- stage 6: transpose memPre -> mpreT ----
                mpreT = bigB.tile([128, 2, 32, 128], bf16, tag="mpreT")
                for s in range(2):
                    for cg in range(4):
                        tp = ps_tp.tile([128, 1024], bf16, tag="tp")
                        for c8 in range(8):
                            c = cg * 8 + c8
                            nc.tensor.transpose(
                                tp[:, c8 * 128:(c8 + 1) * 128],
                                mpre[:, s, c * 128:(c + 1) * 128], ident[:])
                        if cg % 2 == 0:
                            nc.vector.tensor_copy(
                                mpreT[:, s, cg * 8:(cg + 1) * 8, :], tp[:])
                        else:
                            nc.scalar.copy(
                                mpreT[:, s, cg * 8:(cg + 1) * 8, :], tp[:])

                # ---- stage 7: mv GEMMs ----
                ezt = ps_mv.tile([128, 2, 128], f32, tag="mvez")
                avt = ps_mv.tile([64, 2, 128], f32, tag="mvav")
                ez = ezt[:]
                av = avt[:]
                for c in range(32):
                    nc.tensor.matmul(ez, w_ez[:, c, :], mpreT[:, :, c, :],
                                     start=(c == 0), stop=(c == 31))
                for c in range(32):
                    nc.tensor.matmul(av, w_amv[:, c, :], mpreT[:, :, c, :],
                                     start=(c == 0), stop=(c == 31))
                emvT = sml.tile([64, 2, 128], bf16, tag="emvT")
                nc.scalar.activation(emvT[:], ez[0:64], Act.Sigmoid, bias=b_emv)
                amvT = sml.tile([64, 2, 128], bf16, tag="amvT")
                nc.scalar.activation(amvT[:], av, Act.Tanh, bias=b_amv)
                # drain zmv out of psum early so the mv bank frees quickly
                zmv = sml.tile([64, 2, 128], bf16, tag="zmv")
                nc.scalar.activation(zmv[:], ez[64:128], Act.Identity,
                                     bias=b_zmv)


                # ---- stage 8: small epilogue chain ([f, (s,b)]) ----
                wz = ps_sml.tile([128, 2, 128], f32, tag="psml")
                nc.tensor.matmul(wz[:], w_ewz[:], c0T[:], start=True, stop=True)
                ecT = sml.tile([64, 2, 128], bf16, tag="ecT")
                nc.scalar.activation(ecT[:], wz[0:64], Act.Sigmoid, bias=b_e)
                esum = sml.tile([64, 2, 128], bf16, tag="esum")
                nc.vector.tensor_tensor(esum[:], ecT[:], emvT[:], op=Alu.add)
                eT = sml.tile([64, 2, 128], bf16, tag="esum")
                nc.scalar.activation(eT[:], esum[:], Act.Sigmoid)
                zc = sml.tile([64, 2, 128], bf16, tag="zc")
                nc.scalar.activation(zc[:], wz[64:128], Act.Identity, bias=b_z)
                zsum = sml.tile([64, 2, 128], bf16, tag="zc")
                nc.vector.tensor_tensor(zsum[:], zmv[:], zc[:], op=Alu.add)
                ztT = sml.tile([64, 2, 128], bf16, tag="ztT")
                nc.scalar.activation(ztT[:], zsum[:], Act.Sigmoid)
                za = ps_sml.tile([64, 2, 128], f32, tag="psml")
                nc.tensor.matmul(za[:], w_za[:], ztT[:], start=True, stop=True)
                zaT = sml.tile([64, 2, 128], bf16, tag="zaT")
                nc.scalar.activation(zaT[:], za[:], Act.Tanh, bias=b_za)
                asum = sml.tile([64, 2, 128], bf16, tag="asum")
                nc.vector.tensor_tensor(asum[:], zaT[:], amvT[:], op=Alu.add)
                aT = sml.tile([64, 2, 128], bf16, tag="asum")
                nc.scalar.activation(aT[:], asum[:], Act.Tanh)

                # transpose eT/aT -> natural [128(b), s, 64(dv)]
                e_nat = sml.tile([128, 2, 64], bf16, tag="e_nat")
                a_nat = sml.tile([128, 2, 64], bf16, tag="a_nat")
                for s in range(2):
                    te = ps_tp.tile([128, 128], bf16, tag="tp")
                    nc.tensor.transpose(te[:, 0:64], eT[:, s, :],
                                        ident[0:64, 0:64])
                    nc.tensor.transpose(te[:, 64:128], aT[:, s, :],
                                        ident[0:64, 0:64])
                    nc.vector.tensor_copy(e_nat[:, s, :], te[:, 0:64])
                    nc.vector.tensor_copy(a_nat[:, s, :], te[:, 64:128])

                # ---- stage 10: combine  new = mpre + wbig*(abig - mpre*ebig) ----
                w2 = sml.tile([128, 2, 128], bf16, tag="w2")
                nc.vector.tensor_copy(
                    w2[:].rearrange("p s (m r) -> p s m r", r=2),
                    w_nat_all[:, t, :, :].unsqueeze(3)
                    .broadcast_to([128, 2, 64, 2]))
                out = poolmo.tile([128, 2, F], bf16, tag="mem")  # alias mem slots
                for s in range(2):
                    scr = scrp.tile([128, F], bf16, tag="scr")
                    mp = mpre[:, s].rearrange("p (m d) -> p m d", m=64)
                    t1 = scr[:].rearrange("p (m d) -> p m d", m=64)
                    ebig = e_nat[:, s, :].unsqueeze(1).broadcast_to([128, 64, 64])
                    abig = a_nat[:, s, :].unsqueeze(1).broadcast_to([128, 64, 64])
                    # wbig via pair-duplicated w2 so innermost AP step stays 1:
                    # view [p, m, 32, 2]; w2 bcast over the 32 pair groups.
                    w4 = (w2[:, s, :]
                          .rearrange("p (m r) -> p m r", r=2)
                          .unsqueeze(2).broadcast_to([128, 64, 32, 2]))
                    # P1: t1 = mpre * ebig         (DVE)
                    nc.vector.tensor_tensor(t1, mp, ebig, op=Alu.mult)
                    # P2: t1 = abig - t1           (DVE)
                    nc.vector.tensor_tensor(t1, abig, t1, op=Alu.subtract)
                    # P3: t1 = t1 * wbig           (DVE, 2x via pair trick)
                    t1v = scr[:].rearrange("p (m g r) -> p m g r", m=64, r=2)
                    nc.vector.tensor_tensor(t1v, t1v, w4, op=Alu.mult)
                    # P4: out = mpre + t1          (DVE)
                    nc.vector.tensor_tensor(
                        out[:, s].rearrange("p (m d) -> p m d", m=64),
                        mp, t1, op=Alu.add)

                # ---- store (SWDGE cast bf16 -> fp32) ----
                nc.gpsimd.dma_start(out_r[t], out[:])

            def whole():
                w_nat_all = pro.tile([128, NT, 2, 64], bf16, tag="w_nat_all")
                prologue(w_nat_all)
                loaded = load_tile(0)
                st = stage_a2(0, stage_a1(0, loaded))
                for t in range(NT):
                    st_next = None
                    if t + 1 < NT:
                        nxt = load_tile(t + 1)
                        st_next = stage_a1(t + 1, nxt)
                    stage_b(t, w_nat_all, st)
                    if st_next is not None:
                        st_next = stage_a2(t + 1, st_next)
                    st = st_next

            if iters == 1:
                whole()
            else:
                with tc.For_i(0, iters, 1):
                    whole()

    nc.compile()
    return nc


def _get_nc(b_core, iters, with_bm1):
    key = (b_core, iters, with_bm1)
    if key not in _BUILD_CACHE:
        _BUILD_CACHE[key] = _build(b_core, iters, with_bm1)
    return _BUILD_CACHE[key]


def _prep_weights(inputs):
    bf = ml_dtypes.bfloat16
    wc0 = np.ascontiguousarray(
        inputs["Wc0"].reshape(32, 128, 128).transpose(1, 0, 2).reshape(128, -1)
    ).astype(bf)
    wez_full = np.concatenate([inputs["Wemv"], inputs["Wzmv"]], axis=1)
    wez = np.ascontiguousarray(
        wez_full.reshape(32, 128, 128).transpose(1, 0, 2).reshape(128, -1)
    ).astype(bf)
    wamv = np.ascontiguousarray(
        inputs["Wamv"].reshape(32, 128, 64).transpose(1, 0, 2).reshape(128, -1)
    ).astype(bf)
    wewz = np.concatenate([inputs["We"], inputs["Wz"]], axis=1).astype(bf)
    wm1 = inputs["Wm1"].astype(bf)
    wza = inputs["Wza"].astype(bf)
    mkt = np.ascontiguousarray(inputs["memory_key"].T).astype(bf)

    biasv = np.zeros((128, 8), np.float32)
    biasv[:, 0] = inputs["bc0"]
    biasv[0:64, 1] = inputs["be"]
    biasv[0:64, 2] = inputs["bz"]
    biasv[0:64, 3] = inputs["bemv"]
    biasv[0:64, 4] = inputs["bzmv"]
    biasv[0:64, 5] = inputs["bamv"]
    biasv[0:64, 6] = inputs["bza"]

    w = dict(wc0=wc0, wm1=wm1, wez=wez, wamv=wamv, wewz=wewz, wza=wza,
             mkt=mkt, biasv=biasv)
    with_bm1 = bool(np.any(inputs["bm1"]))
    if with_bm1:
        w["bm1r"] = inputs["bm1"].reshape(1, F).astype(bf)
    return w, with_bm1


def _make_in_maps(inputs, b_core):
    wdict, _ = _prep_weights(inputs)
    mem = np.ascontiguousarray(inputs["memory_value"].reshape(-1, F))
    qa = np.ascontiguousarray(inputs["control_qa"])
    ck = np.ascontiguousarray(inputs["control_key"])
    in_maps = []
    for c in range(N_CORES):
        sl = slice(c * b_core, (c + 1) * b_core)
        in_maps.append(dict(mem=mem[sl], qa=qa[sl], ck=ck[sl], **wdict))
    return in_maps


def kernel(**inputs):
    from concourse import bass_utils
    inputs = {k: np.asarray(v) for k, v in inputs.items()}
    _, with_bm1 = _prep_weights(inputs)
    nc = _get_nc(B_CORE, 1, with_bm1)
    in_maps = _make_in_maps(inputs, B_CORE)
    res = bass_utils.run_bass_kernel_spmd(nc, in_maps, core_ids=list(range(N_CORES)))
    out = np.concatenate([r["out"] for r in res.results], axis=0)
    return out.reshape(B, M, DV).astype(np.float32)


# revision 34
# speedup vs baseline: 1.8936x; 1.0064x over previous
# Trainium2 Bass kernel for nn_MEMORY_34986803593776 (scatter_memory).
#
# Math (per sample b):
#   w        = softmax(ck @ mk^T)                             [M]
#   c0       = qa * sigmoid(mem0 @ Wc0 + bc0)                 [DQA]
#   gate     = sigmoid(c0 @ Wm1 + bm1)                        [M*DV]
#   memPre   = mem0 * gate                                    [M*DV]
#   erase    = sig(sig(c0@We+be) + sig(memPre@Wemv+bemv))     [DV]
#   zt       = sig((c0@Wz+bz) + (memPre@Wzmv+bzmv))           [DV]
#   add      = tanh(tanh(zt@Wza+bza) + tanh(memPre@Wamv+bamv))[DV]
#   new      = memPre*(1 - w[m]*erase[dv]) + w[m]*add[dv]     [M,DV]
#
# Sharding: pure data parallel over batch B=16384 across 8 cores (2048/core).
# On-chip: natural [b, f] layout for elementwise; PE-transposed [f, b]
# chunks feed the f-contraction GEMMs. Big elementwise in bf16.
# Softmax for all tiles is hoisted into a prologue (one ACT-table switch).

import numpy as np
import ml_dtypes

B = 16384
M = 64
DV = 64
DK = 64
DQA = 128
F = M * DV  # 4096
N_CORES = 8
B_CORE = B // N_CORES  # 2048

_BUILD_CACHE = {}


def _build(b_core, iters, with_bm1):
    """Build and compile the single-core Bass program."""
    import concourse.tile as tile
    import concourse.bacc as bacc
    import concourse.mybir as mybir
    from concourse import masks
    from contextlib import ExitStack

    f32 = mybir.dt.float32
    bf16 = mybir.dt.bfloat16
    Alu = mybir.AluOpType
    Act = mybir.ActivationFunctionType

    NT = b_core // 256  # tiles of 256 samples
    assert b_core % 256 == 0

    nc = bacc.Bacc("TRN2", target_bir_lowering=False, debug=False,
                   num_devices=N_CORES)

    # ---- DRAM tensors (host-prepped layouts) ----
    d_mem = nc.dram_tensor("mem", (b_core, F), f32, kind="ExternalInput")
    d_qa = nc.dram_tensor("qa", (b_core, DQA), f32, kind="ExternalInput")
    d_ck = nc.dram_tensor("ck", (b_core, DK), f32, kind="ExternalInput")
    d_wc0 = nc.dram_tensor("wc0", (128, 32 * 128), bf16, kind="ExternalInput")
    d_wm1 = nc.dram_tensor("wm1", (128, F), bf16, kind="ExternalInput")
    d_wez = nc.dram_tensor("wez", (128, 32 * 128), bf16, kind="ExternalInput")
    d_wamv = nc.dram_tensor("wamv", (128, 32 * 64), bf16, kind="ExternalInput")
    d_wewz = nc.dram_tensor("wewz", (128, 128), bf16, kind="ExternalInput")
    d_wza = nc.dram_tensor("wza", (DV, DV), bf16, kind="ExternalInput")
    d_mkt = nc.dram_tensor("mkt", (DK, M), bf16, kind="ExternalInput")
    d_bias = nc.dram_tensor("biasv", (128, 8), f32, kind="ExternalInput")
    if with_bm1:
        d_bm1 = nc.dram_tensor("bm1r", (1, F), bf16, kind="ExternalInput")
    d_out = nc.dram_tensor("out", (b_core, F), f32, kind="ExternalOutput")

    mem_r = d_mem.ap().rearrange("(t s p) f -> t p s f", p=128, s=2)
    qa_r = d_qa.ap().rearrange("(t s p) f -> t p s f", p=128, s=2)
    ck_r = d_ck.ap().rearrange("(t s p) f -> t p s f", p=128, s=2)
    out_r = d_out.ap().rearrange("(t s p) f -> t p s f", p=128, s=2)

    with tile.TileContext(nc) as tc:
        with ExitStack() as ctx:
            wpool = ctx.enter_context(tc.tile_pool(name="wpool", bufs=1))
            poolmo = ctx.enter_context(tc.tile_pool(name="poolmo", bufs=3))
            scrp = ctx.enter_context(tc.tile_pool(name="scrp", bufs=2))
            bigA = ctx.enter_context(tc.tile_pool(name="bigA", bufs=2))
            bigB = ctx.enter_context(tc.tile_pool(name="bigB", bufs=1))
            sml = ctx.enter_context(tc.tile_pool(name="sml", bufs=2))
            pro = ctx.enter_context(tc.tile_pool(name="pro", bufs=1))
            ps_tp = ctx.enter_context(tc.tile_pool(name="ps_tp", bufs=2, space="PSUM"))
            ps_gate = ctx.enter_context(tc.tile_pool(name="ps_gate", bufs=2, space="PSUM"))
            ps_sml = ctx.enter_context(tc.tile_pool(name="ps_sml", bufs=1, space="PSUM"))
            ps_mv = ctx.enter_context(tc.tile_pool(name="ps_mv", bufs=1, space="PSUM"))

            # ---- weights into SBUF (once) ----
            w_c0 = wpool.tile([128, 32, 128], bf16, tag="w_c0")
            nc.sync.dma_start(w_c0[:], d_wc0.ap().rearrange("k (c q) -> k c q", c=32))
            w_m1 = wpool.tile([128, F], bf16, tag="w_m1")
            nc.sync.dma_start(w_m1[:], d_wm1.ap())
            w_ez = wpool.tile([128, 32, 128], bf16, tag="w_ez")
            nc.sync.dma_start(w_ez[:], d_wez.ap().rearrange("k (c q) -> k c q", c=32))
            w_amv = wpool.tile([128, 32, 64], bf16, tag="w_amv")
            nc.sync.dma_start(w_amv[:], d_wamv.ap().rearrange("k (c q) -> k c q", c=32))
            w_ewz = wpool.tile([128, 128], bf16, tag="w_ewz")
            nc.sync.dma_start(w_ewz[:], d_wewz.ap())
            w_za = wpool.tile([DV, DV], bf16, tag="w_za")
            nc.sync.dma_start(w_za[:], d_wza.ap())
            w_mkt = wpool.tile([DK, M], bf16, tag="w_mkt")
            nc.sync.dma_start(w_mkt[:], d_mkt.ap())
            biasv = wpool.tile([128, 8], f32, tag="biasv")
            nc.sync.dma_start(biasv[:], d_bias.ap())
            if with_bm1:
                bm1r = wpool.tile([1, F], bf16, tag="bm1r")
                nc.sync.dma_start(bm1r[:], d_bm1.ap())
                ones1 = wpool.tile([1, 128], bf16, tag="ones1")
                nc.vector.memset(ones1[:], 1.0)
            ident = wpool.tile([128, 128], bf16, tag="ident")
            masks.make_identity(nc, ident[:])

            bc0 = biasv[:, 0:1]
            b_e = biasv[0:64, 1:2]
            b_z = biasv[0:64, 2:3]
            b_emv = biasv[0:64, 3:4]
            b_zmv = biasv[0:64, 4:5]
            b_amv = biasv[0:64, 5:6]
            b_za = biasv[0:64, 6:7]

            def prologue(w_nat_all):
                """Softmax for all tiles: w = softmax(ck @ mk^T), natural [b, m].
                Also materialize w2 (pair-duplicated w) for the combine."""
                for t in range(NT):
                    ck = sml.tile([128, 2, DK], bf16, tag="qa")
                    nc.gpsimd.dma_start(ck[:], ck_r[t])
                    ckT = sml.tile([64, 2, 128], bf16, tag="ecT")
                    for s in range(2):
                        tk = ps_tp.tile([128, 128], bf16, tag="tp")
                        nc.tensor.transpose(tk[0:64, :], ck[:, s, :], ident[:])
                        nc.vector.tensor_copy(ckT[:, s, :], tk[0:64, :])
                    lg = ps_sml.tile([128, 2, 64], f32, tag="psml")
                    for s in range(2):
                        nc.tensor.matmul(lg[:, s], ckT[:, s, :], w_mkt[:],
                                         start=True, stop=True)
                    for s in range(2):
                        mx = sml.tile([128, 1], f32, tag="mx")
                        nc.vector.tensor_reduce(mx[:], lg[:, s],
                                                mybir.AxisListType.X,
                                                Alu.max, negate=True)
                        exv = sml.tile([128, 64], f32, tag="exv")
                        nc.scalar.activation(exv[:], lg[:, s], Act.Exp, bias=mx[:])
                        sm = sml.tile([128, 1], f32, tag="sm")
                        nc.vector.tensor_reduce(sm[:], exv[:],
                                                mybir.AxisListType.X, Alu.add)
                        nc.vector.reciprocal(sm[:], sm[:])
                        nc.vector.tensor_scalar_mul(w_nat_all[:, t, s, :], exv[:],
                                                    sm[:])


            def load_tile(t):
                qa = sml.tile([128, 2, DQA], bf16, tag="qa")
                nc.gpsimd.dma_start(qa[:], qa_r[t])
                mem = poolmo.tile([128, 2, F], bf16, tag="mem")
                nc.gpsimd.dma_start(mem[:], mem_r[t])
                return mem, qa

            def stage_a1(t, loaded):
                mem, qa = loaded

                # ---- stage 2: transpose mem -> memT chunks ([f, b]) ----
                memT = bigB.tile([128, 2, 32, 128], bf16, tag="memT")
                for s in range(2):
                    for cg in range(4):
                        tp = ps_tp.tile([128, 1024], bf16, tag="tp")
                        for c8 in range(8):
                            c = cg * 8 + c8
                            nc.tensor.transpose(
                                tp[:, c8 * 128:(c8 + 1) * 128],
                                mem[:, s, c * 128:(c + 1) * 128], ident[:])
                        nc.scalar.copy(memT[:, s, cg * 8:(cg + 1) * 8, :], tp[:])

                # ---- stage 3: content0 (layout [q, (s,b)]) ----
                c0ps = ps_mv.tile([128, 2, 128], f32, tag="mvc0")
                for c in range(32):
                    nc.tensor.matmul(c0ps[:], w_c0[:, c, :], memT[:, :, c, :],
                                     start=(c == 0), stop=(c == 31))
                c0s = sml.tile([128, 2, 128], bf16, tag="c0s")
                nc.scalar.activation(c0s[:], c0ps[:], Act.Sigmoid, bias=bc0)
                qaT = sml.tile([128, 2, 128], bf16, tag="qaT")
                for s in range(2):
                    tq = ps_tp.tile([128, 128], bf16, tag="tp")
                    nc.tensor.transpose(tq[:], qa[:, s, :], ident[:])
                    nc.vector.tensor_copy(qaT[:, s, :], tq[:])
                return dict(mem=mem, memT=memT, c0s=c0s, qaT=qaT)

            def stage_a2(t, st):
                c0T = sml.tile([128, 2, 128], bf16, tag="c0T")
                nc.vector.tensor_tensor(c0T[:], st["c0s"][:], st["qaT"][:],
                                        op=Alu.mult)

                # ---- stage 4: gate (natural layout [b, f]) ----
                gate = bigA.tile([128, 2, F], bf16, tag="gate")
                for s in range(2):
                    for h in range(8):
                        gps = ps_gate.tile([128, 512], f32, tag="gate")
                        nsl = slice(h * 512, (h + 1) * 512)
                        nc.tensor.matmul(gps[:], c0T[:, s, :], w_m1[:, nsl],
                                         start=True, stop=not with_bm1)
                        if with_bm1:
                            nc.tensor.matmul(gps[:], ones1[:], bm1r[:, nsl],
                                             start=False, stop=True)
                        nc.scalar.activation(gate[:, s, nsl], gps[:], Act.Sigmoid)

                st["c0T"] = c0T
                st["gate"] = gate
                return st

            def stage_b1(t, st):
                mem, gate = st["mem"], st["gate"]
                # ---- stage 5: memPre = mem * gate ----
                mpre = bigA.tile([128, 2, F], bf16, tag="mpre")
                for s in range(2):
                    nc.vector.tensor_tensor(mpre[:, s], mem[:, s], gate[:, s],
                                            op=Alu.mult)

                # ---- stage 6: transpose memPre -> mpreT ----
                mpreT = bigB.tile([128, 2, 32, 128], bf16, tag="mpreT")
                for s in range(2):
                    for cg in range(4):
                        tp = ps_tp.tile([128, 1024], bf16, tag="tp")
                        for c8 in range(8):
                            c = cg * 8 + c8
                            nc.tensor.transpose(
                                tp[:, c8 * 128:(c8 + 1) * 128],
                                mpre[:, s, c * 128:(c + 1) * 128], ident[:])
                        if cg % 2 == 0:
                            nc.vector.tensor_copy(
                                mpreT[:, s, cg * 8:(cg + 1) * 8, :], tp[:])
                        else:
                            nc.scalar.copy(
                                mpreT[:, s, cg * 8:(cg + 1) * 8, :], tp[:])

                st["mpre"] = mpre
                st["mpreT"] = mpreT
                return st

            def stage_b2(t, w_nat_all, st):
                mem, memT, c0T = st["mem"], st["memT"], st["c0T"]
                mpre, mpreT = st["mpre"], st["mpreT"]
                # ---- stage 7: mv GEMMs ----
                ezt = ps_mv.tile([128, 2, 128], f32, tag="mvez")
                avt = ps_mv.tile([64, 2, 128], f32, tag="mvav")
                ez = ezt[:]
                av = avt[:]
                for c in range(32):
                    nc.tensor.matmul(ez[:], w_ez[:, c, :], mpreT[:, :, c, :],
                                     start=(c == 0), stop=(c == 31))
                for c in range(32):
                    nc.tensor.matmul(av[:], w_amv[:, c, :], mpreT[:, :, c, :],
                                     start=(c == 0), stop=(c == 31))
                emvT = sml.tile([64, 2, 128], bf16, tag="emvT")
                nc.scalar.activation(emvT[:], ez[0:64], Act.Sigmoid, bias=b_emv)
                amvT = sml.tile([64, 2, 128], bf16, tag="amvT")
                nc.scalar.activation(amvT[:], av[:], Act.Tanh, bias=b_amv)
                # drain zmv out of psum early so the mv bank frees quickly
                zmv = sml.tile([64, 2, 128], bf16, tag="zmv")
                nc.scalar.activation(zmv[:], ez[64:128], Act.Identity,
                                     bias=b_zmv)


                # ---- stage 8: small epilogue chain ([f, (s,b)]) ----
                wz = ps_sml.tile([128, 2, 128], f32, tag="psml")
                nc.tensor.matmul(wz[:], w_ewz[:], c0T[:], start=True, stop=True)
                ecT = sml.tile([64, 2, 128], bf16, tag="ecT")
                nc.scalar.activation(ecT[:], wz[0:64], Act.Sigmoid, bias=b_e)
                esum = sml.tile([64, 2, 128], bf16, tag="esum")
                nc.vector.tensor_tensor(esum[:], ecT[:], emvT[:], op=Alu.add)
                eT = sml.tile([64, 2, 128], bf16, tag="esum")
                nc.scalar.activation(eT[:], esum[:], Act.Sigmoid)
                zc = sml.tile([64, 2, 128], bf16, tag="zc")
                nc.scalar.activation(zc[:], wz[64:128], Act.Identity, bias=b_z)
                zsum = sml.tile([64, 2, 128], bf16, tag="zc")
                nc.vector.tensor_tensor(zsum[:], zmv[:], zc[:], op=Alu.add)
                ztT = sml.tile([64, 2, 128], bf16, tag="ztT")
                nc.scalar.activation(ztT[:], zsum[:], Act.Sigmoid)
                za = ps_sml.tile([64, 2, 128], f32, tag="psml")
                nc.tensor.matmul(za[:], w_za[:], ztT[:], start=True, stop=True)
                zaT = sml.tile([64, 2, 128], bf16, tag="zaT")
                nc.scalar.activation(zaT[:], za[:], Act.Tanh, bias=b_za)
                asum = sml.tile([64, 2, 128], bf16, tag="asum")
                nc.vector.tensor_tensor(asum[:], zaT[:], amvT[:], op=Alu.add)
                aT = sml.tile([64, 2, 128], bf16, tag="asum")
                nc.scalar.activation(aT[:], asum[:], Act.Tanh)

                # transpose eT/aT -> natural [128(b), s, 64(dv)]
                e_nat = sml.tile([128, 2, 64], bf16, tag="e_nat")
                a_nat = sml.tile([128, 2, 64], bf16, tag="a_nat")
                for s in range(2):
                    te = ps_tp.tile([128, 128], bf16, tag="tp")
                    nc.tensor.transpose(te[:, 0:64], eT[:, s, :],
                                        ident[0:64, 0:64])
                    nc.tensor.transpose(te[:, 64:128], aT[:, s, :],
                                        ident[0:64, 0:64])
                    nc.vector.tensor_copy(e_nat[:, s, :], te[:, 0:64])
                    nc.vector.tensor_copy(a_nat[:, s, :], te[:, 64:128])

                # ---- stage 10: combine  new = mpre + wbig*(abig - mpre*ebig) ----
                w2 = sml.tile([128, 2, 128], bf16, tag="w2")
                nc.vector.tensor_copy(
                    w2[:].rearrange("p s (m r) -> p s m r", r=2),
                    w_nat_all[:, t, :, :].unsqueeze(3)
                    .broadcast_to([128, 2, 64, 2]))
                out = poolmo.tile([128, 2, F], bf16, tag="mem")  # alias mem slots
                for s in range(2):
                    scr = scrp.tile([128, F], bf16, tag="scr")
                    mp = mpre[:, s].rearrange("p (m d) -> p m d", m=64)
                    t1 = scr[:].rearrange("p (m d) -> p m d", m=64)
                    ebig = e_nat[:, s, :].unsqueeze(1).broadcast_to([128, 64, 64])
                    abig = a_nat[:, s, :].unsqueeze(1).broadcast_to([128, 64, 64])
                    # wbig via pair-duplicated w2 so innermost AP step stays 1:
                    # view [p, m, 32, 2]; w2 bcast over the 32 pair groups.
                    w4 = (w2[:, s, :]
                          .rearrange("p (m r) -> p m r", r=2)
                          .unsqueeze(2).broadcast_to([128, 64, 32, 2]))
                    # P1: t1 = mpre * ebig         (DVE)
                    nc.vector.tensor_tensor(t1, mp, ebig, op=Alu.mult)
                    # P2: t1 = abig - t1           (DVE)
                    nc.vector.tensor_tensor(t1, abig, t1, op=Alu.subtract)
                    # P3: t1 = t1 * wbig           (DVE, 2x via pair trick)
                    t1v = scr[:].rearrange("p (m g r) -> p m g r", m=64, r=2)
                    nc.vector.tensor_tensor(t1v, t1v, w4, op=Alu.mult)
                    # P4: out = mpre + t1          (DVE)
                    nc.vector.tensor_tensor(
                        out[:, s].rearrange("p (m d) -> p m d", m=64),
                        mp, t1, op=Alu.add)

                # ---- store (SWDGE cast bf16 -> fp32) ----
                nc.gpsimd.dma_start(out_r[t], out[:])

            def whole():
                w_nat_all = pro.tile([128, NT, 2, 64], bf16, tag="w_nat_all")
                prologue(w_nat_all)
                loaded = load_tile(0)
                st = stage_a2(0, stage_a1(0, loaded))
                for t in range(NT):
                    st_next = None
                    if t + 1 < NT:
                        nxt = load_tile(t + 1)
                        st_next = stage_a1(t + 1, nxt)
                    st = stage_b1(t, st)
                    stage_b2(t, w_nat_all, st)
                    if st_next is not None:
                        st_next = stage_a2(t + 1, st_next)
                    st = st_next

            if iters == 1:
                whole()
            else:
                with tc.For_i(0, iters, 1):
                    whole()

    nc.compile()
    return nc


def _get_nc(b_core, iters, with_bm1):
    key = (b_core, iters, with_bm1)
    if key not in _BUILD_CACHE:
        _BUILD_CACHE[key] = _build(b_core, iters, with_bm1)
    return _BUILD_CACHE[key]


def _prep_weights(inputs):
    bf = ml_dtypes.bfloat16
    wc0 = np.ascontiguousarray(
        inputs["Wc0"].reshape(32, 128, 128).transpose(1, 0, 2).reshape(128, -1)
    ).astype(bf)
    wez_full = np.concatenate([inputs["Wemv"], inputs["Wzmv"]], axis=1)
    wez = np.ascontiguousarray(
        wez_full.reshape(32, 128, 128).transpose(1, 0, 2).reshape(128, -1)
    ).astype(bf)
    wamv = np.ascontiguousarray(
        inputs["Wamv"].reshape(32, 128, 64).transpose(1, 0, 2).reshape(128, -1)
    ).astype(bf)
    wewz = np.concatenate([inputs["We"], inputs["Wz"]], axis=1).astype(bf)
    wm1 = inputs["Wm1"].astype(bf)
    wza = inputs["Wza"].astype(bf)
    mkt = np.ascontiguousarray(inputs["memory_key"].T).astype(bf)

    biasv = np.zeros((128, 8), np.float32)
    biasv[:, 0] = inputs["bc0"]
    biasv[0:64, 1] = inputs["be"]
    biasv[0:64, 2] = inputs["bz"]
    biasv[0:64, 3] = inputs["bemv"]
    biasv[0:64, 4] = inputs["bzmv"]
    biasv[0:64, 5] = inputs["bamv"]
    biasv[0:64, 6] = inputs["bza"]

    w = dict(wc0=wc0, wm1=wm1, wez=wez, wamv=wamv, wewz=wewz, wza=wza,
             mkt=mkt, biasv=biasv)
    with_bm1 = bool(np.any(inputs["bm1"]))
    if with_bm1:
        w["bm1r"] = inputs["bm1"].reshape(1, F).astype(bf)
    return w, with_bm1


def _make_in_maps(inputs, b_core):
    wdict, _ = _prep_weights(inputs)
    mem = np.ascontiguousarray(inputs["memory_value"].reshape(-1, F))
    qa = np.ascontiguousarray(inputs["control_qa"])
    ck = np.ascontiguousarray(inputs["control_key"])
    in_maps = []
    for c in range(N_CORES):
        sl = slice(c * b_core, (c + 1) * b_core)
        in_maps.append(dict(mem=mem[sl], qa=qa[sl], ck=ck[sl], **wdict))
    return in_maps


def kernel(**inputs):
    from concourse import bass_utils
    inputs = {k: np.asarray(v) for k, v in inputs.items()}
    _, with_bm1 = _prep_weights(inputs)
    nc = _get_nc(B_CORE, 1, with_bm1)
    in_maps = _make_in_maps(inputs, B_CORE)
    res = bass_utils.run_bass_kernel_spmd(nc, in_maps, core_ids=list(range(N_CORES)))
    out = np.concatenate([r["out"] for r in res.results], axis=0)
    return out.reshape(B, M, DV).astype(np.float32)


# revision 38
# speedup vs baseline: 1.8978x; 1.0022x over previous
# Trainium2 Bass kernel for nn_MEMORY_34986803593776 (scatter_memory).
#
# Math (per sample b):
#   w        = softmax(ck @ mk^T)                             [M]
#   c0       = qa * sigmoid(mem0 @ Wc0 + bc0)                 [DQA]
#   gate     = sigmoid(c0 @ Wm1 + bm1)                        [M*DV]
#   memPre   = mem0 * gate                                    [M*DV]
#   erase    = sig(sig(c0@We+be) + sig(memPre@Wemv+bemv))     [DV]
#   zt       = sig((c0@Wz+bz) + (memPre@Wzmv+bzmv))           [DV]
#   add      = tanh(tanh(zt@Wza+bza) + tanh(memPre@Wamv+bamv))[DV]
#   new      = memPre*(1 - w[m]*erase[dv]) + w[m]*add[dv]     [M,DV]
#
# Sharding: pure data parallel over batch B=16384 across 8 cores (2048/core).
# On-chip: natural [b, f] layout for elementwise; PE-transposed [f, b]
# chunks feed the f-contraction GEMMs. Big elementwise in bf16.
# Softmax for all tiles is hoisted into a prologue (one ACT-table switch).

import numpy as np
import ml_dtypes

B = 16384
M = 64
DV = 64
DK = 64
DQA = 128
F = M * DV  # 4096
N_CORES = 8
B_CORE = B // N_CORES  # 2048

_BUILD_CACHE = {}


def _build(b_core, iters, with_bm1):
    """Build and compile the single-core Bass program."""
    import concourse.tile as tile
    import concourse.bacc as bacc
    import concourse.mybir as mybir
    from concourse import masks
    from contextlib import ExitStack

    f32 = mybir.dt.float32
    bf16 = mybir.dt.bfloat16
    Alu = mybir.AluOpType
    Act = mybir.ActivationFunctionType

    NT = b_core // 256  # tiles of 256 samples
    assert b_core % 256 == 0

    nc = bacc.Bacc("TRN2", target_bir_lowering=False, debug=False,
                   num_devices=N_CORES)

    # ---- DRAM tensors (host-prepped layouts) ----
    d_mem = nc.dram_tensor("mem", (b_core, F), f32, kind="ExternalInput")
    d_qa = nc.dram_tensor("qa", (b_core, DQA), f32, kind="ExternalInput")
    d_ck = nc.dram_tensor("ck", (b_core, DK), f32, kind="ExternalInput")
    d_wc0 = nc.dram_tensor("wc0", (128, 32 * 128), bf16, kind="ExternalInput")
    d_wm1 = nc.dram_tensor("wm1", (128, F), bf16, kind="ExternalInput")
    d_wez = nc.dram_tensor("wez", (128, 32 * 128), bf16, kind="ExternalInput")
    d_wamv = nc.dram_tensor("wamv", (128, 32 * 64), bf16, kind="ExternalInput")
    d_wewz = nc.dram_tensor("wewz", (128, 128), bf16, kind="ExternalInput")
    d_wza = nc.dram_tensor("wza", (DV, DV), bf16, kind="ExternalInput")
    d_mkt = nc.dram_tensor("mkt", (DK, M), bf16, kind="ExternalInput")
    d_bias = nc.dram_tensor("biasv", (128, 8), f32, kind="ExternalInput")
    if with_bm1:
        d_bm1 = nc.dram_tensor("bm1r", (1, F), bf16, kind="ExternalInput")
    d_out = nc.dram_tensor("out", (b_core, F), f32, kind="ExternalOutput")

    mem_r = d_mem.ap().rearrange("(t s p) f -> t p s f", p=128, s=2)
    qa_r = d_qa.ap().rearrange("(t s p) f -> t p s f", p=128, s=2)
    ck_r = d_ck.ap().rearrange("(t s p) f -> t p s f", p=128, s=2)
    out_r = d_out.ap().rearrange("(t s p) f -> t p s f", p=128, s=2)

    with tile.TileContext(nc) as tc:
        with ExitStack() as ctx:
            wpool = ctx.enter_context(tc.tile_pool(name="wpool", bufs=1))
            poolmo = ctx.enter_context(tc.tile_pool(name="poolmo", bufs=3))
            scrp = ctx.enter_context(tc.tile_pool(name="scrp", bufs=2))
            bigA = ctx.enter_context(tc.tile_pool(name="bigA", bufs=2))
            bigB = ctx.enter_context(tc.tile_pool(name="bigB", bufs=1))
            sml = ctx.enter_context(tc.tile_pool(name="sml", bufs=2))
            pro = ctx.enter_context(tc.tile_pool(name="pro", bufs=1))
            ps_tp = ctx.enter_context(tc.tile_pool(name="ps_tp", bufs=2, space="PSUM"))
            ps_gate = ctx.enter_context(tc.tile_pool(name="ps_gate", bufs=2, space="PSUM"))
            ps_sml = ctx.enter_context(tc.tile_pool(name="ps_sml", bufs=1, space="PSUM"))
            ps_mv = ctx.enter_context(tc.tile_pool(name="ps_mv", bufs=1, space="PSUM"))

            # ---- weights into SBUF (once) ----
            w_c0 = wpool.tile([128, 32, 128], bf16, tag="w_c0")
            nc.sync.dma_start(w_c0[:], d_wc0.ap().rearrange("k (c q) -> k c q", c=32))
            w_m1 = wpool.tile([128, F], bf16, tag="w_m1")
            nc.sync.dma_start(w_m1[:], d_wm1.ap())
            w_ez = wpool.tile([128, 32, 128], bf16, tag="w_ez")
            nc.sync.dma_start(w_ez[:], d_wez.ap().rearrange("k (c q) -> k c q", c=32))
            w_amv = wpool.tile([128, 32, 64], bf16, tag="w_amv")
            nc.sync.dma_start(w_amv[:], d_wamv.ap().rearrange("k (c q) -> k c q", c=32))
            w_ewz = wpool.tile([128, 128], bf16, tag="w_ewz")
            nc.sync.dma_start(w_ewz[:], d_wewz.ap())
            w_za = wpool.tile([DV, DV], bf16, tag="w_za")
            nc.sync.dma_start(w_za[:], d_wza.ap())
            w_mkt = wpool.tile([DK, M], bf16, tag="w_mkt")
            nc.sync.dma_start(w_mkt[:], d_mkt.ap())
            biasv = wpool.tile([128, 8], f32, tag="biasv")
            nc.sync.dma_start(biasv[:], d_bias.ap())
            if with_bm1:
                bm1r = wpool.tile([1, F], bf16, tag="bm1r")
                nc.sync.dma_start(bm1r[:], d_bm1.ap())
                ones1 = wpool.tile([1, 128], bf16, tag="ones1")
                nc.vector.memset(ones1[:], 1.0)
            ident = wpool.tile([128, 128], bf16, tag="ident")
            masks.make_identity(nc, ident[:])

            bc0 = biasv[:, 0:1]
            b_e = biasv[0:64, 1:2]
            b_z = biasv[0:64, 2:3]
            b_emv = biasv[0:64, 3:4]
            b_zmv = biasv[0:64, 4:5]
            b_amv = biasv[0:64, 5:6]
            b_za = biasv[0:64, 6:7]

            def prologue(w_nat_all, ck_all):
                """Softmax for all tiles: w = softmax(ck @ mk^T), natural [b, m]."""
                for t in range(NT):
                    ck = ck_all[:, t]
                    ckT = sml.tile([64, 2, 128], bf16, tag="ecT")
                    for s in range(2):
                        tk = ps_tp.tile([128, 128], bf16, tag="tp")
                        nc.tensor.transpose(tk[0:64, :], ck[:, s, :], ident[:])
                        nc.vector.tensor_copy(ckT[:, s, :], tk[0:64, :])
                    lg = ps_sml.tile([128, 2, 64], f32, tag="psml")
                    for s in range(2):
                        nc.tensor.matmul(lg[:, s], ckT[:, s, :], w_mkt[:],
                                         start=True, stop=True)
                    for s in range(2):
                        mx = sml.tile([128, 1], f32, tag="mx")
                        nc.vector.tensor_reduce(mx[:], lg[:, s],
                                                mybir.AxisListType.X,
                                                Alu.max, negate=True)
                        exv = sml.tile([128, 64], f32, tag="exv")
                        nc.scalar.activation(exv[:], lg[:, s], Act.Exp, bias=mx[:])
                        sm = sml.tile([128, 1], f32, tag="sm")
                        nc.vector.tensor_reduce(sm[:], exv[:],
                                                mybir.AxisListType.X, Alu.add)
                        nc.vector.reciprocal(sm[:], sm[:])
                        nc.vector.tensor_scalar_mul(w_nat_all[:, t, s, :], exv[:],
                                                    sm[:])


            def load_tile(t):
                qa = sml.tile([128, 2, DQA], bf16, tag="qa")
                nc.gpsimd.dma_start(qa[:], qa_r[t])
                mem = poolmo.tile([128, 2, F], bf16, tag="mem")
                nc.gpsimd.dma_start(mem[:], mem_r[t])
                return mem, qa

            def stage_a1(t, loaded):
                mem, qa = loaded

                # ---- stage 2: transpose mem -> memT chunks ([f, b]) ----
                memT = bigB.tile([128, 2, 32, 128], bf16, tag="memT")
                for s in range(2):
                    for cg in range(4):
                        tp = ps_tp.tile([128, 1024], bf16, tag="tp")
                        for c8 in range(8):
                            c = cg * 8 + c8
                            nc.tensor.transpose(
                                tp[:, c8 * 128:(c8 + 1) * 128],
                                mem[:, s, c * 128:(c + 1) * 128], ident[:])
                        nc.scalar.copy(memT[:, s, cg * 8:(cg + 1) * 8, :], tp[:])

                # ---- stage 3: content0 (layout [q, (s,b)]) ----
                c0ps = ps_mv.tile([128, 2, 128], f32, tag="mvc0")
                for c in range(32):
                    nc.tensor.matmul(c0ps[:], w_c0[:, c, :], memT[:, :, c, :],
                                     start=(c == 0), stop=(c == 31))
                c0s = sml.tile([128, 2, 128], bf16, tag="c0s")
                nc.scalar.activation(c0s[:], c0ps[:], Act.Sigmoid, bias=bc0)
                qaT = sml.tile([128, 2, 128], bf16, tag="qaT")
                for s in range(2):
                    tq = ps_tp.tile([128, 128], bf16, tag="tp")
                    nc.tensor.transpose(tq[:], qa[:, s, :], ident[:])
                    nc.vector.tensor_copy(qaT[:, s, :], tq[:])
                return dict(mem=mem, memT=memT, c0s=c0s, qaT=qaT)

            def stage_a2(t, st):
                c0T = sml.tile([128, 2, 128], bf16, tag="c0T")
                nc.vector.tensor_tensor(c0T[:], st["c0s"][:], st["qaT"][:],
                                        op=Alu.mult)

                # ---- stage 4: gate (natural layout [b, f]) ----
                gate = bigA.tile([128, 2, F], bf16, tag="gate")
                for s in range(2):
                    for h in range(8):
                        gps = ps_gate.tile([128, 512], f32, tag="gate")
                        nsl = slice(h * 512, (h + 1) * 512)
                        nc.tensor.matmul(gps[:], c0T[:, s, :], w_m1[:, nsl],
                                         start=True, stop=not with_bm1)
                        if with_bm1:
                            nc.tensor.matmul(gps[:], ones1[:], bm1r[:, nsl],
                                             start=False, stop=True)
                        nc.scalar.activation(gate[:, s, nsl], gps[:], Act.Sigmoid)

                st["c0T"] = c0T
                st["gate"] = gate
                return st

            def stage_b1(t, st):
                mem, gate = st["mem"], st["gate"]
                # ---- stage 5: memPre = mem * gate ----
                mpre = bigA.tile([128, 2, F], bf16, tag="mpre")
                for s in range(2):
                    nc.vector.tensor_tensor(mpre[:, s], mem[:, s], gate[:, s],
                                            op=Alu.mult)

                # ---- stage 6: transpose memPre -> mpreT ----
                mpreT = bigB.tile([128, 2, 32, 128], bf16, tag="mpreT")
                for s in range(2):
                    for cg in range(4):
                        tp = ps_tp.tile([128, 1024], bf16, tag="tp")
                        for c8 in range(8):
                            c = cg * 8 + c8
                            nc.tensor.transpose(
                                tp[:, c8 * 128:(c8 + 1) * 128],
                                mpre[:, s, c * 128:(c + 1) * 128], ident[:])
                        if cg % 2 == 0:
                            nc.vector.tensor_copy(
                                mpreT[:, s, cg * 8:(cg + 1) * 8, :], tp[:])
                        else:
                            nc.scalar.copy(
                                mpreT[:, s, cg * 8:(cg + 1) * 8, :], tp[:])

                st["mpre"] = mpre
                st["mpreT"] = mpreT
                return st

            def stage_b2(t, w_nat_all, st):
                mem, memT, c0T = st["mem"], st["memT"], st["c0T"]
                mpre, mpreT = st["mpre"], st["mpreT"]
                # ---- stage 7: mv GEMMs ----
                ezt = ps_mv.tile([128, 2, 128], f32, tag="mvez")
                avt = ps_mv.tile([64, 2, 128], f32, tag="mvav")
                ez = ezt[:]
                av = avt[:]
                for c in range(32):
                    nc.tensor.matmul(ez[:], w_ez[:, c, :], mpreT[:, :, c, :],
                                     start=(c == 0), stop=(c == 31))
                for c in range(32):
                    nc.tensor.matmul(av[:], w_amv[:, c, :], mpreT[:, :, c, :],
                                     start=(c == 0), stop=(c == 31))
                emvT = sml.tile([64, 2, 128], bf16, tag="emvT")
                nc.scalar.activation(emvT[:], ez[0:64], Act.Sigmoid, bias=b_emv)
                amvT = sml.tile([64, 2, 128], bf16, tag="amvT")
                nc.scalar.activation(amvT[:], av[:], Act.Tanh, bias=b_amv)
                # drain zmv out of psum early so the mv bank frees quickly
                zmv = sml.tile([64, 2, 128], bf16, tag="zmv")
                nc.scalar.activation(zmv[:], ez[64:128], Act.Identity,
                                     bias=b_zmv)


                # ---- stage 8: small epilogue chain ([f, (s,b)]) ----
                wz = ps_sml.tile([128, 2, 128], f32, tag="psml")
                nc.tensor.matmul(wz[:], w_ewz[:], c0T[:], start=True, stop=True)
                ecT = sml.tile([64, 2, 128], bf16, tag="ecT")
                nc.scalar.activation(ecT[:], wz[0:64], Act.Sigmoid, bias=b_e)
                esum = sml.tile([64, 2, 128], bf16, tag="esum")
                nc.vector.tensor_tensor(esum[:], ecT[:], emvT[:], op=Alu.add)
                eT = sml.tile([64, 2, 128], bf16, tag="esum")
                nc.scalar.activation(eT[:], esum[:], Act.Sigmoid)
                zc = sml.tile([64, 2, 128], bf16, tag="zc")
                nc.scalar.activation(zc[:], wz[64:128], Act.Identity, bias=b_z)
                zsum = sml.tile([64, 2, 128], bf16, tag="zc")
                nc.vector.tensor_tensor(zsum[:], zmv[:], zc[:], op=Alu.add)
                ztT = sml.tile([64, 2, 128], bf16, tag="ecT")
                nc.scalar.activation(ztT[:], zsum[:], Act.Sigmoid)
                za = ps_sml.tile([64, 2, 128], f32, tag="psml")
                nc.tensor.matmul(za[:], w_za[:], ztT[:], start=True, stop=True)
                zaT = sml.tile([64, 2, 128], bf16, tag="zmv")
                nc.scalar.activation(zaT[:], za[:], Act.Tanh, bias=b_za)
                asum = sml.tile([64, 2, 128], bf16, tag="asum")
                nc.vector.tensor_tensor(asum[:], zaT[:], amvT[:], op=Alu.add)
                aT = sml.tile([64, 2, 128], bf16, tag="asum")
                nc.scalar.activation(aT[:], asum[:], Act.Tanh)

                # transpose eT/aT -> natural [128(b), s, 64(dv)]
                e_nat = sml.tile([128, 2, 64], bf16, tag="e_nat")
                a_nat = sml.tile([128, 2, 64], bf16, tag="a_nat")
                for s in range(2):
                    te = ps_tp.tile([128, 128], bf16, tag="tp")
                    nc.tensor.transpose(te[:, 0:64], eT[:, s, :],
                                        ident[0:64, 0:64])
                    nc.tensor.transpose(te[:, 64:128], aT[:, s, :],
                                        ident[0:64, 0:64])
                    nc.vector.tensor_copy(e_nat[:, s, :], te[:, 0:64])
                    nc.vector.tensor_copy(a_nat[:, s, :], te[:, 64:128])

                # ---- stage 10: combine  new = mpre + wbig*(abig - mpre*ebig) ----
                w2 = sml.tile([128, 2, 128], bf16, tag="w2")
                nc.vector.tensor_copy(
                    w2[:].rearrange("p s (m r) -> p s m r", r=2),
                    w_nat_all[:, t, :, :].unsqueeze(3)
                    .broadcast_to([128, 2, 64, 2]))
                out = poolmo.tile([128, 2, F], bf16, tag="mem")  # alias mem slots
                for s in range(2):
                    scr = scrp.tile([128, F], bf16, tag="scr")
                    mp = mpre[:, s].rearrange("p (m d) -> p m d", m=64)
                    t1 = scr[:].rearrange("p (m d) -> p m d", m=64)
                    ebig = e_nat[:, s, :].unsqueeze(1).broadcast_to([128, 64, 64])
                    abig = a_nat[:, s, :].unsqueeze(1).broadcast_to([128, 64, 64])
                    # wbig via pair-duplicated w2 so innermost AP step stays 1:
                    # view [p, m, 32, 2]; w2 bcast over the 32 pair groups.
                    w4 = (w2[:, s, :]
                          .rearrange("p (m r) -> p m r", r=2)
                          .unsqueeze(2).broadcast_to([128, 64, 32, 2]))
                    # P1: t1 = mpre * ebig         (DVE)
                    nc.vector.tensor_tensor(t1, mp, ebig, op=Alu.mult)
                    # P2: t1 = abig - t1           (DVE)
                    nc.vector.tensor_tensor(t1, abig, t1, op=Alu.subtract)
                    # P3: t1 = t1 * wbig           (DVE, 2x via pair trick)
                    t1v = scr[:].rearrange("p (m g r) -> p m g r", m=64, r=2)
                    nc.vector.tensor_tensor(t1v, t1v, w4, op=Alu.mult)
                    # P4: out = mpre + t1          (DVE)
                    nc.vector.tensor_tensor(
                        out[:, s].rearrange("p (m d) -> p m d", m=64),
                        mp, t1, op=Alu.add)

                # ---- store (SWDGE cast bf16 -> fp32) ----
                nc.gpsimd.dma_start(out_r[t], out[:])

            def whole():
                w_nat_all = pro.tile([128, NT, 2, 64], bf16, tag="w_nat_all")
                ck_all = pro.tile([128, NT, 2, DK], bf16, tag="ck_all")
                nc.gpsimd.dma_start(ck_all[:],
                                    ck_r.transpose([1, 0, 2, 3]))
                loaded = load_tile(0)
                prologue(w_nat_all, ck_all)
                st = stage_a2(0, stage_a1(0, loaded))
                for t in range(NT):
                    st_next = None
                    if t + 1 < NT:
                        nxt = load_tile(t + 1)
                        st_next = stage_a1(t + 1, nxt)
                    st = stage_b1(t, st)
                    stage_b2(t, w_nat_all, st)
                    if st_next is not None:
                        st_next = stage_a2(t + 1, st_next)
                    st = st_next

            if iters == 1:
                whole()
            else:
                with tc.For_i(0, iters, 1,
                              hint_engines=(mybir.EngineType.PE,
                                            mybir.EngineType.DVE,
                                            mybir.EngineType.Activation,
                                            mybir.EngineType.Pool,
                                            mybir.EngineType.SP)):
                    whole()

    nc.compile()
    return nc


def _get_nc(b_core, iters, with_bm1):
    key = (b_core, iters, with_bm1)
    if key not in _BUILD_CACHE:
        _BUILD_CACHE[key] = _build(b_core, iters, with_bm1)
    return _BUILD_CACHE[key]


def _prep_weights(inputs):
    bf = ml_dtypes.bfloat16
    wc0 = np.ascontiguousarray(
        inputs["Wc0"].reshape(32, 128, 128).transpose(1, 0, 2).reshape(128, -1)
    ).astype(bf)
    wez_full = np.concatenate([inputs["Wemv"], inputs["Wzmv"]], axis=1)
    wez = np.ascontiguousarray(
        wez_full.reshape(32, 128, 128).transpose(1, 0, 2).reshape(128, -1)
    ).astype(bf)
    wamv = np.ascontiguousarray(
        inputs["Wamv"].reshape(32, 128, 64).transpose(1, 0, 2).reshape(128, -1)
    ).astype(bf)
    wewz = np.concatenate([inputs["We"], inputs["Wz"]], axis=1).astype(bf)
    wm1 = inputs["Wm1"].astype(bf)
    wza = inputs["Wza"].astype(bf)
    mkt = np.ascontiguousarray(inputs["memory_key"].T).astype(bf)

    biasv = np.zeros((128, 8), np.float32)
    biasv[:, 0] = inputs["bc0"]
    biasv[0:64, 1] = inputs["be"]
    biasv[0:64, 2] = inputs["bz"]
    biasv[0:64, 3] = inputs["bemv"]
    biasv[0:64, 4] = inputs["bzmv"]
    biasv[0:64, 5] = inputs["bamv"]
    biasv[0:64, 6] = inputs["bza"]

    w = dict(wc0=wc0, wm1=wm1, wez=wez, wamv=wamv, wewz=wewz, wza=wza,
             mkt=mkt, biasv=biasv)
    with_bm1 = bool(np.any(inputs["bm1"]))
    if with_bm1:
        w["bm1r"] = inputs["bm1"].reshape(1, F).astype(bf)
    return w, with_bm1


def _make_in_maps(inputs, b_core):
    wdict, _ = _prep_weights(inputs)
    mem = np.ascontiguousarray(inputs["memory_value"].reshape(-1, F))
    qa = np.ascontiguousarray(inputs["control_qa"])
    ck = np.ascontiguousarray(inputs["control_key"])
    in_maps = []
    for c in range(N_CORES):
        sl = slice(c * b_core, (c + 1) * b_core)
        in_maps.append(dict(mem=mem[sl], qa=qa[sl], ck=ck[sl], **wdict))
    return in_maps


def kernel(**inputs):
    from concourse import bass_utils
    inputs = {k: np.asarray(v) for k, v in inputs.items()}
    _, with_bm1 = _prep_weights(inputs)
    nc = _get_nc(B_CORE, 1, with_bm1)
    in_maps = _make_in_maps(inputs, B_CORE)
    res = bass_utils.run_bass_kernel_spmd(nc, in_maps, core_ids=list(range(N_CORES)))
    out = np.concatenate([r["out"] for r in res.results], axis=0)
    return out.reshape(B, M, DV).astype(np.float32)


# revision 39
# speedup vs baseline: 1.9190x; 1.0112x over previous
# Trainium2 Bass kernel for nn_MEMORY_34986803593776 (scatter_memory).
#
# Math (per sample b):
#   w        = softmax(ck @ mk^T)                             [M]
#   c0       = qa * sigmoid(mem0 @ Wc0 + bc0)                 [DQA]
#   gate     = sigmoid(c0 @ Wm1 + bm1)                        [M*DV]
#   memPre   = mem0 * gate                                    [M*DV]
#   erase    = sig(sig(c0@We+be) + sig(memPre@Wemv+bemv))     [DV]
#   zt       = sig((c0@Wz+bz) + (memPre@Wzmv+bzmv))           [DV]
#   add      = tanh(tanh(zt@Wza+bza) + tanh(memPre@Wamv+bamv))[DV]
#   new      = memPre*(1 - w[m]*erase[dv]) + w[m]*add[dv]     [M,DV]
#
# Sharding: pure data parallel over batch B=16384 across 8 cores (2048/core).
# On-chip: natural [b, f] layout for elementwise; PE-transposed [f, b]
# chunks feed the f-contraction GEMMs. Big elementwise in bf16.
# Softmax for all tiles is hoisted into a prologue (one ACT-table switch).

import numpy as np
import ml_dtypes

B = 16384
M = 64
DV = 64
DK = 64
DQA = 128
F = M * DV  # 4096
N_CORES = 8
B_CORE = B // N_CORES  # 2048

_BUILD_CACHE = {}


def _build(b_core, iters, with_bm1):
    """Build and compile the single-core Bass program."""
    import concourse.tile as tile
    import concourse.bacc as bacc
    import concourse.mybir as mybir
    from concourse import masks
    from contextlib import ExitStack

    f32 = mybir.dt.float32
    bf16 = mybir.dt.bfloat16
    Alu = mybir.AluOpType
    Act = mybir.ActivationFunctionType

    NT = b_core // 256  # tiles of 256 samples
    assert b_core % 256 == 0

    nc = bacc.Bacc("TRN2", target_bir_lowering=False, debug=False,
                   num_devices=N_CORES)

    # ---- DRAM tensors (host-prepped layouts) ----
    d_mem = nc.dram_tensor("mem", (b_core, F), f32, kind="ExternalInput")
    d_qa = nc.dram_tensor("qa", (b_core, DQA), f32, kind="ExternalInput")
    d_ck = nc.dram_tensor("ck", (b_core, DK), f32, kind="ExternalInput")
    d_wc0 = nc.dram_tensor("wc0", (128, 32 * 128), bf16, kind="ExternalInput")
    d_wm1 = nc.dram_tensor("wm1", (128, F), bf16, kind="ExternalInput")
    d_wez = nc.dram_tensor("wez", (128, 32 * 128), bf16, kind="ExternalInput")
    d_wamv = nc.dram_tensor("wamv", (128, 32 * 64), bf16, kind="ExternalInput")
    d_wewz = nc.dram_tensor("wewz", (128, 128), bf16, kind="ExternalInput")
    d_wza = nc.dram_tensor("wza", (DV, DV), bf16, kind="ExternalInput")
    d_mkt = nc.dram_tensor("mkt", (DK, M), bf16, kind="ExternalInput")
    d_bias = nc.dram_tensor("biasv", (128, 8), f32, kind="ExternalInput")
    if with_bm1:
        d_bm1 = nc.dram_tensor("bm1r", (1, F), bf16, kind="ExternalInput")
    d_out = nc.dram_tensor("out", (b_core, F), f32, kind="ExternalOutput")

    mem_r = d_mem.ap().rearrange("(t s p) f -> t p s f", p=128, s=2)
    qa_r = d_qa.ap().rearrange("(t s p) f -> t p s f", p=128, s=2)
    ck_r = d_ck.ap().rearrange("(t s p) f -> t p s f", p=128, s=2)
    out_r = d_out.ap().rearrange("(t s p) f -> t p s f", p=128, s=2)

    with tile.TileContext(nc) as tc:
        with ExitStack() as ctx:
            wpool = ctx.enter_context(tc.tile_pool(name="wpool", bufs=1))
            poolmo = ctx.enter_context(tc.tile_pool(name="poolmo", bufs=3))
            scrp = ctx.enter_context(tc.tile_pool(name="scrp", bufs=2))
            bigA = ctx.enter_context(tc.tile_pool(name="bigA", bufs=2))
            bigB = ctx.enter_context(tc.tile_pool(name="bigB", bufs=1))
            sml = ctx.enter_context(tc.tile_pool(name="sml", bufs=2))
            pro = ctx.enter_context(tc.tile_pool(name="pro", bufs=1))
            ps_tp = ctx.enter_context(tc.tile_pool(name="ps_tp", bufs=2, space="PSUM"))
            ps_gate = ctx.enter_context(tc.tile_pool(name="ps_gate", bufs=2, space="PSUM"))
            ps_sml = ctx.enter_context(tc.tile_pool(name="ps_sml", bufs=1, space="PSUM"))
            ps_mv = ctx.enter_context(tc.tile_pool(name="ps_mv", bufs=1, space="PSUM"))

            # ---- weights into SBUF (once) ----
            w_c0 = wpool.tile([128, 32, 128], bf16, tag="w_c0")
            nc.sync.dma_start(w_c0[:], d_wc0.ap().rearrange("k (c q) -> k c q", c=32))
            w_m1 = wpool.tile([128, F], bf16, tag="w_m1")
            nc.sync.dma_start(w_m1[:], d_wm1.ap())
            w_ez = wpool.tile([128, 32, 128], bf16, tag="w_ez")
            nc.sync.dma_start(w_ez[:], d_wez.ap().rearrange("k (c q) -> k c q", c=32))
            w_amv = wpool.tile([128, 32, 64], bf16, tag="w_amv")
            nc.sync.dma_start(w_amv[:], d_wamv.ap().rearrange("k (c q) -> k c q", c=32))
            w_ewz = wpool.tile([128, 128], bf16, tag="w_ewz")
            nc.sync.dma_start(w_ewz[:], d_wewz.ap())
            w_za = wpool.tile([DV, DV], bf16, tag="w_za")
            nc.sync.dma_start(w_za[:], d_wza.ap())
            w_mkt = wpool.tile([DK, M], bf16, tag="w_mkt")
            nc.sync.dma_start(w_mkt[:], d_mkt.ap())
            biasv = wpool.tile([128, 8], f32, tag="biasv")
            nc.sync.dma_start(biasv[:], d_bias.ap())
            if with_bm1:
                bm1r = wpool.tile([1, F], bf16, tag="bm1r")
                nc.sync.dma_start(bm1r[:], d_bm1.ap())
                ones1 = wpool.tile([1, 128], bf16, tag="ones1")
                nc.vector.memset(ones1[:], 1.0)
            ident = wpool.tile([128, 128], bf16, tag="ident")
            masks.make_identity(nc, ident[:])

            bc0 = biasv[:, 0:1]
            b_e = biasv[0:64, 1:2]
            b_z = biasv[0:64, 2:3]
            b_emv = biasv[0:64, 3:4]
            b_zmv = biasv[0:64, 4:5]
            b_amv = biasv[0:64, 5:6]
            b_za = biasv[0:64, 6:7]

            def prologue(w_nat_all, ck_all):
                """Softmax for all tiles: w = softmax(ck @ mk^T), natural [b, m]."""
                for t in range(NT):
                    ck = ck_all[:, t]
                    ckT = sml.tile([64, 2, 128], bf16, tag="ecT")
                    for s in range(2):
                        tk = ps_tp.tile([128, 128], bf16, tag="tp")
                        nc.tensor.transpose(tk[0:64, :], ck[:, s, :], ident[:])
                        nc.vector.tensor_copy(ckT[:, s, :], tk[0:64, :])
                    lg = ps_sml.tile([128, 2, 64], f32, tag="psml")
                    for s in range(2):
                        nc.tensor.matmul(lg[:, s], ckT[:, s, :], w_mkt[:],
                                         start=True, stop=True)
                    for s in range(2):
                        mx = sml.tile([128, 1], f32, tag="mx")
                        nc.vector.tensor_reduce(mx[:], lg[:, s],
                                                mybir.AxisListType.X,
                                                Alu.max, negate=True)
                        exv = sml.tile([128, 64], f32, tag="exv")
                        nc.scalar.activation(exv[:], lg[:, s], Act.Exp, bias=mx[:])
                        sm = sml.tile([128, 1], f32, tag="sm")
                        nc.vector.tensor_reduce(sm[:], exv[:],
                                                mybir.AxisListType.X, Alu.add)
                        nc.vector.reciprocal(sm[:], sm[:])
                        nc.vector.tensor_scalar_mul(w_nat_all[:, t, s, :], exv[:],
                                                    sm[:])


            def load_tile(t):
                qa = sml.tile([128, 2, DQA], bf16, tag="qa")
                nc.gpsimd.dma_start(qa[:], qa_r[t])
                mem = poolmo.tile([128, 2, F], bf16, tag="mem")
                nc.gpsimd.dma_start(mem[:], mem_r[t])
                return mem, qa

            def stage_a1(t, loaded):
                mem, qa = loaded

                # ---- stage 2: transpose mem -> memT chunks ([f, b]) ----
                memT = bigB.tile([128, 2, 32, 128], bf16, tag="memT")
                for s in range(2):
                    for cg in range(4):
                        tp = ps_tp.tile([128, 1024], bf16, tag="tp")
                        for c8 in range(8):
                            c = cg * 8 + c8
                            nc.tensor.transpose(
                                tp[:, c8 * 128:(c8 + 1) * 128],
                                mem[:, s, c * 128:(c + 1) * 128], ident[:])
                        nc.scalar.copy(memT[:, s, cg * 8:(cg + 1) * 8, :], tp[:])

                # ---- stage 3: content0 (layout [q, (s,b)]) ----
                c0ps = ps_mv.tile([128, 2, 128], f32, tag="mvc0")
                for c in range(32):
                    nc.tensor.matmul(c0ps[:], w_c0[:, c, :], memT[:, :, c, :],
                                     start=(c == 0), stop=(c == 31))
                c0s = sml.tile([128, 2, 128], bf16, tag="c0s")
                nc.scalar.activation(c0s[:], c0ps[:], Act.Sigmoid, bias=bc0)
                qaT = sml.tile([128, 2, 128], bf16, tag="qaT")
                for s in range(2):
                    tq = ps_tp.tile([128, 128], bf16, tag="tp")
                    nc.tensor.transpose(tq[:], qa[:, s, :], ident[:])
                    nc.vector.tensor_copy(qaT[:, s, :], tq[:])
                return dict(mem=mem, memT=memT, c0s=c0s, qaT=qaT)

            def stage_a2(t, st):
                c0T = sml.tile([128, 2, 128], bf16, tag="c0T")
                nc.vector.tensor_tensor(c0T[:], st["c0s"][:], st["qaT"][:],
                                        op=Alu.mult)

                # ---- stage 4: gate (natural layout [b, f]) ----
                gate = bigA.tile([128, 2, F], bf16, tag="gate")
                for s in range(2):
                    for h in range(8):
                        gps = ps_gate.tile([128, 512], f32, tag="gate")
                        nsl = slice(h * 512, (h + 1) * 512)
                        nc.tensor.matmul(gps[:], c0T[:, s, :], w_m1[:, nsl],
                                         start=True, stop=not with_bm1)
                        if with_bm1:
                            nc.tensor.matmul(gps[:], ones1[:], bm1r[:, nsl],
                                             start=False, stop=True)
                        nc.scalar.activation(gate[:, s, nsl], gps[:], Act.Sigmoid)

                st["c0T"] = c0T
                st["gate"] = gate
                return st

            def stage_b1(t, st):
                mem, gate = st["mem"], st["gate"]
                # ---- stage 5: memPre = mem * gate ----
                mpre = bigA.tile([128, 2, F], bf16, tag="mpre")
                for s in range(2):
                    nc.vector.tensor_tensor(mpre[:, s], mem[:, s], gate[:, s],
                                            op=Alu.mult)

                # ---- stage 6: transpose memPre -> mpreT ----
                mpreT = bigB.tile([128, 2, 32, 128], bf16, tag="mpreT")
                for s in range(2):
                    for cg in range(4):
                        tp = ps_tp.tile([128, 1024], bf16, tag="tp")
                        for c8 in range(8):
                            c = cg * 8 + c8
                            nc.tensor.transpose(
                                tp[:, c8 * 128:(c8 + 1) * 128],
                                mpre[:, s, c * 128:(c + 1) * 128], ident[:])
                        if cg % 2 == 0:
                            nc.vector.tensor_copy(
                                mpreT[:, s, cg * 8:(cg + 1) * 8, :], tp[:])
                        else:
                            nc.scalar.copy(
                                mpreT[:, s, cg * 8:(cg + 1) * 8, :], tp[:])

                st["mpre"] = mpre
                st["mpreT"] = mpreT
                return st

            def stage_b2(t, w_nat_all, st):
                mem, memT, c0T = st["mem"], st["memT"], st["c0T"]
                mpre, mpreT = st["mpre"], st["mpreT"]
                # ---- stage 7: mv GEMMs ----
                ezt = ps_mv.tile([128, 2, 128], f32, tag="mvez")
                avt = ps_mv.tile([64, 2, 128], f32, tag="mvav")
                ez = ezt[:]
                av = avt[:]
                for c in range(32):
                    nc.tensor.matmul(ez[:], w_ez[:, c, :], mpreT[:, :, c, :],
                                     start=(c == 0), stop=(c == 31))
                for c in range(32):
                    nc.tensor.matmul(av[:], w_amv[:, c, :], mpreT[:, :, c, :],
                                     start=(c == 0), stop=(c == 31))
                emvT = sml.tile([64, 2, 128], bf16, tag="emvT")
                nc.scalar.activation(emvT[:], ez[0:64], Act.Sigmoid, bias=b_emv)
                amvT = sml.tile([64, 2, 128], bf16, tag="amvT")
                nc.scalar.activation(amvT[:], av[:], Act.Tanh, bias=b_amv)
                # drain zmv out of psum early so the mv bank frees quickly
                zmv = sml.tile([64, 2, 128], bf16, tag="zmv")
                nc.scalar.activation(zmv[:], ez[64:128], Act.Identity,
                                     bias=b_zmv)


                # ---- stage 8: small epilogue chain ([f, (s,b)]) ----
                wz = ps_sml.tile([128, 2, 128], f32, tag="psml")
                nc.tensor.matmul(wz[:], w_ewz[:], c0T[:], start=True, stop=True)
                ecT = sml.tile([64, 2, 128], bf16, tag="ecT")
                nc.scalar.activation(ecT[:], wz[0:64], Act.Sigmoid, bias=b_e)
                esum = sml.tile([64, 2, 128], bf16, tag="esum")
                nc.vector.tensor_tensor(esum[:], ecT[:], emvT[:], op=Alu.add)
                eT = sml.tile([64, 2, 128], bf16, tag="esum")
                nc.scalar.activation(eT[:], esum[:], Act.Sigmoid)
                zc = sml.tile([64, 2, 128], bf16, tag="zc")
                nc.scalar.activation(zc[:], wz[64:128], Act.Identity, bias=b_z)
                zsum = sml.tile([64, 2, 128], bf16, tag="zc")
                nc.vector.tensor_tensor(zsum[:], zmv[:], zc[:], op=Alu.add)
                ztT = sml.tile([64, 2, 128], bf16, tag="ecT")
                nc.scalar.activation(ztT[:], zsum[:], Act.Sigmoid)
                za = ps_sml.tile([64, 2, 128], f32, tag="psml")
                nc.tensor.matmul(za[:], w_za[:], ztT[:], start=True, stop=True)
                zaT = sml.tile([64, 2, 128], bf16, tag="zmv")
                nc.scalar.activation(zaT[:], za[:], Act.Tanh, bias=b_za)
                asum = sml.tile([64, 2, 128], bf16, tag="asum")
                nc.vector.tensor_tensor(asum[:], zaT[:], amvT[:], op=Alu.add)
                aT = sml.tile([64, 2, 128], bf16, tag="asum")
                nc.scalar.activation(aT[:], asum[:], Act.Tanh)

                # transpose eT/aT -> natural [128(b), s, 64(dv)]
                e_nat = sml.tile([128, 2, 64], bf16, tag="e_nat")
                a_nat = sml.tile([128, 2, 64], bf16, tag="a_nat")
                for s in range(2):
                    te = ps_tp.tile([128, 128], bf16, tag="tp")
                    nc.tensor.transpose(te[:, 0:64], eT[:, s, :],
                                        ident[0:64, 0:64])
                    nc.tensor.transpose(te[:, 64:128], aT[:, s, :],
                                        ident[0:64, 0:64])
                    nc.vector.tensor_copy(e_nat[:, s, :], te[:, 0:64])
                    nc.vector.tensor_copy(a_nat[:, s, :], te[:, 64:128])

                # ---- stage 10: combine  new = mpre + wbig*(abig - mpre*ebig) ----
                w2 = sml.tile([128, 2, 128], bf16, tag="w2")
                nc.vector.tensor_copy(
                    w2[:].rearrange("p s (m r) -> p s m r", r=2),
                    w_nat_all[:, t, :, :].unsqueeze(3)
                    .broadcast_to([128, 2, 64, 2]))
                out = poolmo.tile([128, 2, F], bf16, tag="mem")  # alias mem slots
                for s in range(2):
                    scr = scrp.tile([128, F], bf16, tag="scr")
                    mp = mpre[:, s].rearrange("p (m d) -> p m d", m=64)
                    t1 = scr[:].rearrange("p (m d) -> p m d", m=64)
                    ebig = e_nat[:, s, :].unsqueeze(1).broadcast_to([128, 64, 64])
                    abig = a_nat[:, s, :].unsqueeze(1).broadcast_to([128, 64, 64])
                    # wbig via pair-duplicated w2 so innermost AP step stays 1:
                    # view [p, m, 32, 2]; w2 bcast over the 32 pair groups.
                    w4 = (w2[:, s, :]
                          .rearrange("p (m r) -> p m r", r=2)
                          .unsqueeze(2).broadcast_to([128, 64, 32, 2]))
                    # P1: t1 = mpre * ebig         (DVE)
                    nc.vector.tensor_tensor(t1, mp, ebig, op=Alu.mult)
                    # P2: t1 = abig - t1           (DVE)
                    nc.vector.tensor_tensor(t1, abig, t1, op=Alu.subtract)
                    # P3: t1 = t1 * wbig           (DVE, 2x via pair trick)
                    t1v = scr[:].rearrange("p (m g r) -> p m g r", m=64, r=2)
                    nc.vector.tensor_tensor(t1v, t1v, w4, op=Alu.mult)
                    # P4: out = mpre + t1          (DVE)
                    nc.vector.tensor_tensor(
                        out[:, s].rearrange("p (m d) -> p m d", m=64),
                        mp, t1, op=Alu.add)

                # ---- store (SWDGE cast bf16 -> fp32) ----
                nc.gpsimd.dma_start(out_r[t], out[:])

            def whole():
                w_nat_all = pro.tile([128, NT, 2, 64], bf16, tag="w_nat_all")
                ck_all = pro.tile([128, NT, 2, DK], bf16, tag="ck_all")
                nc.gpsimd.dma_start(ck_all[:],
                                    ck_r.transpose([1, 0, 2, 3]))
                loaded = load_tile(0)
                prologue(w_nat_all, ck_all)
                st = stage_a1(0, loaded)
                for t in range(NT):
                    st_next = None
                    if t + 1 < NT:
                        nxt = load_tile(t + 1)
                        st_next = stage_a1(t + 1, nxt)
                    st = stage_a2(t, st)
                    st = stage_b1(t, st)
                    stage_b2(t, w_nat_all, st)
                    st = st_next

            if iters == 1:
                whole()
            else:
                with tc.For_i(0, iters, 1,
                              hint_engines=(mybir.EngineType.PE,
                                            mybir.EngineType.DVE,
                                            mybir.EngineType.Activation,
                                            mybir.EngineType.Pool,
                                            mybir.EngineType.SP)):
                    whole()

    nc.compile()
    return nc


def _get_nc(b_core, iters, with_bm1):
    key = (b_core, iters, with_bm1)
    if key not in _BUILD_CACHE:
        _BUILD_CACHE[key] = _build(b_core, iters, with_bm1)
    return _BUILD_CACHE[key]


def _prep_weights(inputs):
    bf = ml_dtypes.bfloat16
    wc0 = np.ascontiguousarray(
        inputs["Wc0"].reshape(32, 128, 128).transpose(1, 0, 2).reshape(128, -1)
    ).astype(bf)
    wez_full = np.concatenate([inputs["Wemv"], inputs["Wzmv"]], axis=1)
    wez = np.ascontiguousarray(
        wez_full.reshape(32, 128, 128).transpose(1, 0, 2).reshape(128, -1)
    ).astype(bf)
    wamv = np.ascontiguousarray(
        inputs["Wamv"].reshape(32, 128, 64).transpose(1, 0, 2).reshape(128, -1)
    ).astype(bf)
    wewz = np.concatenate([inputs["We"], inputs["Wz"]], axis=1).astype(bf)
    wm1 = inputs["Wm1"].astype(bf)
    wza = inputs["Wza"].astype(bf)
    mkt = np.ascontiguousarray(inputs["memory_key"].T).astype(bf)

    biasv = np.zeros((128, 8), np.float32)
    biasv[:, 0] = inputs["bc0"]
    biasv[0:64, 1] = inputs["be"]
    biasv[0:64, 2] = inputs["bz"]
    biasv[0:64, 3] = inputs["bemv"]
    biasv[0:64, 4] = inputs["bzmv"]
    biasv[0:64, 5] = inputs["bamv"]
    biasv[0:64, 6] = inputs["bza"]

    w = dict(wc0=wc0, wm1=wm1, wez=wez, wamv=wamv, wewz=wewz, wza=wza,
             mkt=mkt, biasv=biasv)
    with_bm1 = bool(np.any(inputs["bm1"]))
    if with_bm1:
        w["bm1r"] = inputs["bm1"].reshape(1, F).astype(bf)
    return w, with_bm1


def _make_in_maps(inputs, b_core):
    wdict, _ = _prep_weights(inputs)
    mem = np.ascontiguousarray(inputs["memory_value"].reshape(-1, F))
    qa = np.ascontiguousarray(inputs["control_qa"])
    ck = np.ascontiguousarray(inputs["control_key"])
    in_maps = []
    for c in range(N_CORES):
        sl = slice(c * b_core, (c + 1) * b_core)
        in_maps.append(dict(mem=mem[sl], qa=qa[sl], ck=ck[sl], **wdict))
    return in_maps


def kernel(**inputs):
    from concourse import bass_utils
    inputs = {k: np.asarray(v) for k, v in inputs.items()}
    _, with_bm1 = _prep_weights(inputs)
    nc = _get_nc(B_CORE, 1, with_bm1)
    in_maps = _make_in_maps(inputs, B_CORE)
    res = bass_utils.run_bass_kernel_spmd(nc, in_maps, core_ids=list(range(N_CORES)))
    out = np.concatenate([r["out"] for r in res.results], axis=0)
    return out.reshape(B, M, DV).astype(np.float32)
